# revision 16
# baseline (speedup 1.0000x reference)
"""Trainium2 Bass kernel for nn_Block_47004122087942 (attention + dense top-2 MoE).

Strategy: pure data-parallel over batch (B=8 -> 8 NeuronCores, zero collectives).
Per core: one batch element [T=1024, C=512].
Compute dtype: float32r (fp32 storage, TF32-like matmul @ full PE rate) everywhere
except the gate-logit path, which runs in exact fp32 to reproduce the reference's
discrete top-2 expert selection.

Layout convention: "T" suffix = transposed [channels-on-partitions, tokens-on-free].
matmul(out[M,N], lhsT[K,M], rhs[K,N]) computes lhsT.T @ rhs with K = partitions.
"""
import numpy as np

import concourse.bass as bass
import concourse.tile as tile
from concourse import bacc, mybir
from concourse import bass_utils

P = 128
T = 1024          # tokens per core
C = 512           # channels
H = 8             # heads
D = 64            # head dim
E = 4             # experts
F = 2048          # ffn dim
NT = T // P       # 8 token tiles
NC = C // P       # 4 channel chunks
NF = F // P       # 16 ffn tiles
TCH = 512         # matmul free-dim chunk of tokens
NTCH = T // TCH   # 2
EPS = 1e-5

f32 = mybir.dt.float32
f32r = mybir.dt.float32r
bf16 = mybir.dt.bfloat16


def build_kernel():
    nc = bacc.Bacc("TRN2", target_bir_lowering=False, debug=False, num_devices=8)

    # ---- DRAM parameters (per-core shard views) ----
    x_ext = nc.dram_tensor("x", [T, C], f32, kind="ExternalInput")
    wqT_ext = nc.dram_tensor("wqT", [NC, P, C], f32, kind="ExternalInput")
    wkT_ext = nc.dram_tensor("wkT", [NC, P, C], f32, kind="ExternalInput")
    wvT_ext = nc.dram_tensor("wvT", [NC, P, C], f32, kind="ExternalInput")
    woT_ext = nc.dram_tensor("woT", [NC, P, C], f32, kind="ExternalInput")
    gateT_ext = nc.dram_tensor("gateT", [NC, P, E], f32, kind="ExternalInput")
    wfcT_ext = nc.dram_tensor("wfcT", [E, NC, P, F], f32, kind="ExternalInput")
    wprojT_ext = nc.dram_tensor("wprojT", [E, NF, P, C], f32, kind="ExternalInput")
    maskT_ext = nc.dram_tensor("maskT", [4, P, TCH], f32, kind="ExternalInput")
    iden_ext = nc.dram_tensor("iden", [P, P], f32, kind="ExternalInput")
    y_ext = nc.dram_tensor("y", [T, C], f32, kind="ExternalOutput")
    router_ext = nc.dram_tensor("router", [T, E], f32, kind="ExternalOutput")

    rden_dram = [nc.dram_tensor(f"rden_scratch{kk}", [2, T], f32, kind="Internal")
                 for kk in range(NC)]

    with tile.TileContext(nc) as tc:
        from contextlib import ExitStack
        with ExitStack() as ctx:
            const = ctx.enter_context(tc.tile_pool(name="const", bufs=1))
            persist = ctx.enter_context(tc.tile_pool(name="persist", bufs=1))
            small = ctx.enter_context(tc.tile_pool(name="small", bufs=4))
            ntmp = ctx.enter_context(tc.tile_pool(name="ntmp", bufs=2))
            ps_mm = ctx.enter_context(tc.tile_pool(name="ps_mm", bufs=2, space="PSUM"))
            ps_tp = ctx.enter_context(tc.tile_pool(name="ps_tp", bufs=2, space="PSUM"))

            # ---- constants ----
            iden = const.tile([P, P], f32, name="iden")
            nc.sync.dma_start(out=iden[:], in_=iden_ext.ap())
            eps_t = const.tile([P, 1], f32, name="eps")
            nc.vector.memset(eps_t[:], EPS)
            ones8 = const.tile([P, H], f32, name="ones8")
            nc.vector.memset(ones8[:], 1.0)
            iden_r = const.tile([P, P], f32r, name="idenr")
            nc.gpsimd.dma_start(out=iden_r[:], in_=iden_ext.ap())

            # persistent activations
            x_sb = [persist.tile([P, C], f32, name=f"x{m}") for m in range(NT)]
            attT = [persist.tile([P, T], f32r, name=f"attT{k}") for k in range(NC)]
            xn2T_r = [persist.tile([P, T], bf16, name=f"xn2Tr{k}") for k in range(NC)]
            router_sb = [persist.tile([P, E], f32, name=f"rt{m}") for m in range(NT)]

            for m in range(NT):
                nc.sync.dma_start(out=x_sb[m][:], in_=x_ext.ap()[m * P:(m + 1) * P, :])

            # O-proj / gate weights loaded early so stage D can start the moment
            # attT is normalized (no SBUF-address-reuse dependency on the
            # attention scope close)
            woT_sb = []
            for kk in range(NC):
                w = persist.tile([P, C], f32r, name=f"wo{kk}")
                nc.gpsimd.dma_start(out=w[:], in_=woT_ext.ap()[kk])
                woT_sb.append(w)
            gateT_sb = []
            for kk in range(NC):
                g = persist.tile([P, E], f32, name=f"gate{kk}")
                nc.sync.dma_start(out=g[:], in_=gateT_ext.ap()[kk])
                gateT_sb.append(g)

            def rms_scale(xin, tag):
                """returns [P,1] f32 tile = 1/sqrt(mean(xin^2)+eps)"""
                sq = ntmp.tile([P, C], f32, name="sq")
                nc.vector.tensor_mul(out=sq[:], in0=xin, in1=xin)
                ssq = small.tile([P, 1], f32, name="ssq")
                nc.vector.tensor_reduce(out=ssq[:], in_=sq[:],
                                        axis=mybir.AxisListType.X, op=mybir.AluOpType.add)
                rms = small.tile([P, 1], f32, name="rms")
                nc.scalar.activation(out=rms[:], in_=ssq[:],
                                     func=mybir.ActivationFunctionType.Sqrt,
                                     bias=eps_t[:], scale=1.0 / C)
                rsc = small.tile([P, 1], f32, name="rsc")
                nc.vector.reciprocal(out=rsc[:], in_=rms[:])
                return rsc

            # =========================================================
            # attention scope
            # =========================================================
            with ExitStack() as actx:
                apool = actx.enter_context(tc.tile_pool(name="apool", bufs=1))
                expp = actx.enter_context(tc.tile_pool(name="expp", bufs=3))
                ps_sc = actx.enter_context(tc.tile_pool(name="ps_sc", bufs=2, space="PSUM"))
                ps_pv = actx.enter_context(tc.tile_pool(name="ps_pv", bufs=2, space="PSUM"))

                maskT = []
                for t in range(4):
                    mk = apool.tile([P, TCH], f32r, name=f"mask{t}")
                    nc.gpsimd.dma_start(out=mk[:], in_=maskT_ext.ap()[t])
                    maskT.append(mk)

                # ---- stage A: rms_norm1 + transpose -> xnT (f32r) ----
                bctx = ExitStack()
                bpool = bctx.enter_context(tc.tile_pool(name="bpool", bufs=1))
                xnT = [bpool.tile([P, T], f32r, name=f"xnT{k}") for k in range(NC)]
                for m in range(NT):
                    rsc = rms_scale(x_sb[m][:], f"n1_{m}")
                    xn = ntmp.tile([P, C], f32, name="xn")
                    nc.vector.tensor_scalar_mul(out=xn[:], in0=x_sb[m][:], scalar1=rsc[:])
                    for kk in range(NC):
                        tp = ps_tp.tile([P, P], f32, name="tp")
                        nc.tensor.transpose(tp[:], xn[:, kk * P:(kk + 1) * P], iden[:])
                        nc.vector.tensor_copy(out=xnT[kk][:, m * P:(m + 1) * P], in_=tp[:])

                # ---- stage B: QKV projections ----
                wqT_sb, wkT_sb, wvT_sb = [], [], []
                for kk in range(NC):
                    for lst, ext, nm in ((wqT_sb, wqT_ext, "wq"), (wkT_sb, wkT_ext, "wk"),
                                         (wvT_sb, wvT_ext, "wv")):
                        w = bpool.tile([P, C], f32r, name=f"{nm}{kk}")
                        nc.gpsimd.dma_start(out=w[:], in_=ext.ap()[kk])
                        lst.append(w)

                # q stored per-head, zero-padded to K=128 so the scores matmul
                # can contract over the full 128 partitions (the other head's k
                # rows meet zeros). k stored packed (2 heads per tile).
                zeros512 = apool.tile([P, TCH], f32, name="zeros512")
                nc.vector.memset(zeros512[:], 0.0)
                qz = [apool.tile([P, T], f32r, name=f"qz{h}") for h in range(H)]
                for h in range(H):
                    po = D * (h % 2)
                    zo = D - po  # the half that must be zeroed: 64 if h even else 0
                    for tch in range(NTCH):
                        nc.vector.tensor_copy(
                            out=qz[h][zo:zo + D, tch * TCH:(tch + 1) * TCH],
                            in_=zeros512[0:D, :])
                kT = [apool.tile([P, T], f32r, name=f"kT{o}") for o in range(NC)]
                for o4 in range(NC):
                    for tch in range(NTCH):
                        # K projection (packed)
                        pst = ps_mm.tile([P, TCH], f32, name="mmps")
                        for kk in range(NC):
                            nc.tensor.matmul(
                                pst[:],
                                lhsT=wkT_sb[kk][:, o4 * P:(o4 + 1) * P],
                                rhs=xnT[kk][:, tch * TCH:(tch + 1) * TCH],
                                start=(kk == 0), stop=(kk == NC - 1))
                        nc.vector.tensor_copy(
                            out=kT[o4][:, tch * TCH:(tch + 1) * TCH], in_=pst[:])
                        # Q projection (split per head, placed at matching offset)
                        pst = ps_mm.tile([P, TCH], f32, name="mmps")
                        for kk in range(NC):
                            nc.tensor.matmul(
                                pst[:],
                                lhsT=wqT_sb[kk][:, o4 * P:(o4 + 1) * P],
                                rhs=xnT[kk][:, tch * TCH:(tch + 1) * TCH],
                                start=(kk == 0), stop=(kk == NC - 1))
                        nc.vector.tensor_copy(
                            out=qz[2 * o4][0:D, tch * TCH:(tch + 1) * TCH],
                            in_=pst[0:D, :])
                        nc.vector.tensor_copy(
                            out=qz[2 * o4 + 1][D:P, tch * TCH:(tch + 1) * TCH],
                            in_=pst[D:P, :])

                # V in untransposed layout, interleaved per-head with ones column,
                # zero-padded to a full 128-wide stationary (M=128 matmuls are
                # much faster than M=65 on the fp32r path)
                VW = 128
                zrow = apool.tile([P, VW - D - 1], f32, name="zrow")
                nc.vector.memset(zrow[:], 0.0)
                v_aug = [apool.tile([P, H, VW], f32r, name=f"vaug{m}") for m in range(NT)]
                for m in range(NT):
                    pst = ps_mm.tile([P, C], f32, name="mmps")
                    for kk in range(NC):
                        nc.tensor.matmul(
                            pst[:],
                            lhsT=xnT[kk][:, m * P:(m + 1) * P],
                            rhs=wvT_sb[kk][:],
                            start=(kk == 0), stop=(kk == NC - 1))
                    nc.vector.tensor_copy(
                        out=v_aug[m][:, :, 0:D],
                        in_=pst[:].rearrange("p (h d) -> p h d", h=H))
                    nc.vector.tensor_copy(
                        out=v_aug[m][:, :, D:D + 1].rearrange("p h one -> p (h one)"),
                        in_=ones8[:])
                    for h in range(H):
                        nc.vector.tensor_copy(out=v_aug[m][:, h, D + 1:VW], in_=zrow[:])

                bctx.close()  # free xnT + QKV weights (40KB/partition)
                npool = actx.enter_context(tc.tile_pool(name="npool", bufs=1))

                # ---- stage C: attention, pair-major with pipelined PE emission ----
                for kk2 in range(NC):
                    for hh in range(2):
                        h = 2 * kk2 + hh
                        h2 = h // 2
                        po = D * (h % 2)
                        for ic in range(NTCH):
                            jmax = 4 * (ic + 1)
                            pv = ps_pv.tile([P, TCH], f32, name="pvps")
                            exs = []
                            pending_pv = []
                            for jj in range(jmax):
                                diag = jj >= 4 * ic
                                sc = ps_sc.tile([P, TCH], f32, name="scps")
                                nc.tensor.matmul(
                                    sc[:],
                                    lhsT=kT[h2][:, jj * P:(jj + 1) * P],
                                    rhs=qz[h][:, ic * TCH:(ic + 1) * TCH],
                                    start=True, stop=not diag)
                                if diag:
                                    # accumulate -1e30 additive causal mask on PE
                                    nc.tensor.matmul(
                                        sc[:],
                                        lhsT=iden_r[:],
                                        rhs=maskT[jj - 4 * ic][:],
                                        start=False, stop=True)
                                ex = expp.tile([P, TCH], f32r, name="expT")
                                nc.scalar.activation(out=ex[:], in_=sc[:],
                                                     func=mybir.ActivationFunctionType.Exp)
                                pending_pv.append((jj, ex))
                                # emit pv for the PREVIOUS jj so PE has a score
                                # matmul in flight while ACT computes this exp
                                if len(pending_pv) > 1:
                                    pjj, pex = pending_pv.pop(0)
                                    nc.tensor.matmul(
                                        pv[:, :],
                                        lhsT=v_aug[pjj][:, h, :],
                                        rhs=pex[:],
                                        start=(pjj == 0), stop=False)
                            pjj, pex = pending_pv.pop(0)
                            nc.tensor.matmul(
                                pv[:, :],
                                lhsT=v_aug[pjj][:, h, :],
                                rhs=pex[:],
                                start=(pjj == 0), stop=True)
                            # evict raw attention (f32r) + denominator row
                            nc.vector.tensor_copy(
                                out=attT[h2][po:po + D, ic * TCH:(ic + 1) * TCH],
                                in_=pv[0:D, :])
                            den_t = small.tile([1, TCH], f32, name="den")
                            nc.scalar.copy(out=den_t[:], in_=pv[D:D + 1, :])
                            nc.sync.dma_start(
                                out=rden_dram[kk2].ap()[hh:hh + 1, ic * TCH:(ic + 1) * TCH],
                                in_=den_t[:])
                    # normalize this kk pair immediately (overlaps later heads)
                    bc = npool.tile([P, T], f32, name="bcast")
                    for hh in range(2):
                        srcap = rden_dram[kk2].ap()[hh:hh + 1, :]
                        bsrc = bass.AP(tensor=srcap.tensor, offset=srcap.offset,
                                       ap=[[0, D]] + srcap.ap[1:])
                        nc.sync.dma_start(out=bc[hh * D:(hh + 1) * D, :], in_=bsrc)
                    bc2 = npool.tile([P, T], f32, name="bcrec")
                    scr = npool.tile([P, T], f32, name="bcscr")
                    nc.vector.reciprocal_approx_accurate(out=bc2[:], in_=bc[:], scratch=scr[:])
                    nc.vector.tensor_tensor(out=attT[kk2][:], in0=attT[kk2][:],
                                            in1=bc2[:], op=mybir.AluOpType.mult)
            # attention scope closed (frees ~20MB SBUF)

            # =========================================================
            # stage D: O-projection + residual + norm2 + gate + top-2
            # =========================================================
            with ExitStack() as dctx:
                dpool = dctx.enter_context(tc.tile_pool(name="dpool", bufs=1))
                wfcp = dctx.enter_context(tc.tile_pool(name="wfcp", bufs=2))
                wprp = dctx.enter_context(tc.tile_pool(name="wprp", bufs=1))
                hp = dctx.enter_context(tc.tile_pool(name="hp", bufs=3))
                ps_o = dctx.enter_context(tc.tile_pool(name="ps_o", bufs=4, space="PSUM"))

                def load_expert(e):
                    fc, pr = [], []
                    for kk in range(NC):
                        w = wfcp.tile([P, F], bf16, name=f"wfc{kk}")
                        nc.gpsimd.dma_start(out=w[:], in_=wfcT_ext.ap()[e, kk])
                        fc.append(w)
                    for k in range(NF):
                        w = wprp.tile([P, C], bf16, name=f"wpr{k}")
                        nc.gpsimd.dma_start(out=w[:], in_=wprojT_ext.ap()[e, k])
                        pr.append(w)
                    return fc, pr

                next_w = load_expert(0)
                xn2T_f = [dpool.tile([P, T], f32, name=f"xn2Tf{k}") for k in range(NC)]

                for m in range(NT):
                    pat = ps_mm.tile([P, C], f32, name="mmps")
                    for kk in range(NC):
                        nc.tensor.matmul(
                            pat[:],
                            lhsT=attT[kk][:, m * P:(m + 1) * P],
                            rhs=woT_sb[kk][:],
                            start=(kk == 0), stop=(kk == NC - 1))
                    # x1 = x + att_out (in place into x_sb)
                    nc.vector.tensor_tensor(out=x_sb[m][:], in0=x_sb[m][:], in1=pat[:],
                                            op=mybir.AluOpType.add)
                    rsc = rms_scale(x_sb[m][:], f"n2_{m}")
                    xn2 = ntmp.tile([P, C], f32, name="xn2")
                    nc.vector.tensor_scalar_mul(out=xn2[:], in0=x_sb[m][:], scalar1=rsc[:])
                    for kk in range(NC):
                        tp = ps_tp.tile([P, P], f32, name="tp")
                        nc.tensor.transpose(tp[:], xn2[:, kk * P:(kk + 1) * P], iden[:])
                        nc.vector.tensor_copy(out=xn2T_r[kk][:, m * P:(m + 1) * P], in_=tp[:])
                        nc.scalar.copy(out=xn2T_f[kk][:, m * P:(m + 1) * P], in_=tp[:])

                # gate logits in exact fp32 + top-2 select + normalize
                for m in range(NT):
                    gp = ps_mm.tile([P, C], f32, name="mmps")
                    for kk in range(NC):
                        nc.tensor.matmul(
                            gp[:, 0:E],
                            lhsT=xn2T_f[kk][:, m * P:(m + 1) * P],
                            rhs=gateT_sb[kk][:],
                            start=(kk == 0), stop=(kk == NC - 1))
                    g = gp[:, 0:E]
                    m1 = small.tile([P, 1], f32, name="m1")
                    nc.vector.tensor_reduce(out=m1[:], in_=g, axis=mybir.AxisListType.X,
                                            op=mybir.AluOpType.max)
                    m1n = small.tile([P, 1], f32, name="m1n")
                    nc.vector.tensor_scalar_mul(out=m1n[:], in0=m1[:], scalar1=-1.0)
                    ex = small.tile([P, E], f32, name="gex")
                    nc.scalar.activation(out=ex[:], in_=g,
                                         func=mybir.ActivationFunctionType.Exp,
                                         bias=m1n[:], scale=1.0)
                    eq = small.tile([P, E], f32, name="geq")
                    nc.vector.tensor_scalar(out=eq[:], in0=g, scalar1=m1[:], scalar2=None,
                                            op0=mybir.AluOpType.is_ge)
                    gm = small.tile([P, E], f32, name="ggm")
                    nc.vector.scalar_tensor_tensor(out=gm[:], in0=eq[:], scalar=-1e30,
                                                   in1=g, op0=mybir.AluOpType.mult,
                                                   op1=mybir.AluOpType.add)
                    m2 = small.tile([P, 1], f32, name="m2")
                    nc.vector.tensor_reduce(out=m2[:], in_=gm[:], axis=mybir.AxisListType.X,
                                            op=mybir.AluOpType.max)
                    keep = small.tile([P, E], f32, name="gkeep")
                    nc.vector.tensor_scalar(out=keep[:], in0=g, scalar1=m2[:], scalar2=None,
                                            op0=mybir.AluOpType.is_ge)
                    wun = small.tile([P, E], f32, name="gwun")
                    nc.vector.tensor_tensor(out=wun[:], in0=ex[:], in1=keep[:],
                                            op=mybir.AluOpType.mult)
                    s = small.tile([P, 1], f32, name="gsum")
                    nc.vector.tensor_reduce(out=s[:], in_=wun[:], axis=mybir.AxisListType.X,
                                            op=mybir.AluOpType.add)
                    rs = small.tile([P, 1], f32, name="grs")
                    nc.vector.reciprocal(out=rs[:], in_=s[:])
                    nc.vector.tensor_scalar_mul(out=router_sb[m][:], in0=wun[:], scalar1=rs[:])
                    nc.sync.dma_start(out=router_ext.ap()[m * P:(m + 1) * P, :],
                                      in_=router_sb[m][:])
                # =========================================================
                # stage E: dense MoE experts (pipelined mm1 -> gelu -> mm2)
                # =========================================================
                for e in range(E):
                    wfc_sb, wpr_sb = next_w
                    if e + 1 < E:
                        next_w = load_expert(e + 1)
                    for tch in range(NTCH):
                        po_tiles = [ps_o.tile([P, C], f32, name="moeo")
                                    for i in range(4)]
                        pending = []
                        for k in range(NF):
                            ph = ps_mm.tile([P, TCH], f32, name="mmps")
                            for kk in range(NC):
                                nc.tensor.matmul(
                                    ph[:],
                                    lhsT=wfc_sb[kk][:, k * P:(k + 1) * P],
                                    rhs=xn2T_r[kk][:, tch * TCH:(tch + 1) * TCH],
                                    start=(kk == 0), stop=(kk == NC - 1))
                            hT = hp.tile([P, TCH], bf16, name="hT")
                            nc.scalar.activation(out=hT[:], in_=ph[:],
                                                 func=mybir.ActivationFunctionType.Gelu)
                            pending.append((k, hT))
                            if len(pending) > 1:
                                pk, phT = pending.pop(0)
                                for i in range(4):
                                    nc.tensor.matmul(
                                        po_tiles[i][:],
                                        lhsT=phT[:, i * P:(i + 1) * P],
                                        rhs=wpr_sb[pk][:],
                                        start=(pk == 0), stop=False)
                        pk, phT = pending.pop(0)
                        for i in range(4):
                            nc.tensor.matmul(
                                po_tiles[i][:],
                                lhsT=phT[:, i * P:(i + 1) * P],
                                rhs=wpr_sb[pk][:],
                                start=(pk == 0), stop=True)
                        for i in range(4):
                            m = 4 * tch + i
                            nc.vector.scalar_tensor_tensor(
                                out=x_sb[m][:], in0=po_tiles[i][:],
                                scalar=router_sb[m][:, e:e + 1], in1=x_sb[m][:],
                                op0=mybir.AluOpType.mult, op1=mybir.AluOpType.add)

            for m in range(NT):
                nc.sync.dma_start(out=y_ext.ap()[m * P:(m + 1) * P, :], in_=x_sb[m][:])

    nc.compile()
    return nc


def _prep_inputs(x, ln1_w, ln2_w, wq, wk, wv, wo, gate_w, w_fc, w_proj):
    """Host-side weight preprocessing: fold rms-norm gains + score scale,
    pre-transpose for TensorE (contraction dim on partitions)."""
    f = np.float32
    ln1 = np.asarray(ln1_w, f)
    ln2 = np.asarray(ln2_w, f)
    wqT = np.ascontiguousarray((np.asarray(wq, f) * ln1[None, :]).T).reshape(NC, P, C)
    wkT = np.ascontiguousarray((np.asarray(wk, f) * ln1[None, :]).T / np.sqrt(np.float32(D))).reshape(NC, P, C)
    wvT = np.ascontiguousarray((np.asarray(wv, f) * ln1[None, :]).T).reshape(NC, P, C)
    woT = np.ascontiguousarray(np.asarray(wo, f).T).reshape(NC, P, C)
    gateT = np.ascontiguousarray((np.asarray(gate_w, f) * ln2[None, :]).T).reshape(NC, P, E)
    wfcT = np.stack([
        np.ascontiguousarray((np.asarray(w_fc, f)[e] * ln2[None, :]).T).reshape(NC, P, F)
        for e in range(E)])
    wprojT = np.stack([
        np.ascontiguousarray(np.asarray(w_proj, f)[e].T).reshape(NF, P, C)
        for e in range(E)])
    # additive causal masks in transposed-scores layout: tile covers j in
    # [128t, 128t+128), i in [0, 512) relative; keep (0.0) iff
    # j_local <= i_local - 128t else -1e30 (exp -> 0)
    maskT = np.zeros((4, P, TCH), f)
    jl = np.arange(P)[:, None]
    il = np.arange(TCH)[None, :]
    for t in range(4):
        maskT[t] = np.where(jl <= il - P * t, f(0.0), f(-1e30))
    iden = np.eye(P, dtype=f)
    return dict(wqT=wqT, wkT=wkT, wvT=wvT, woT=woT, gateT=gateT,
                wfcT=wfcT, wprojT=wprojT, maskT=maskT, iden=iden)


_cached_nc = None


def kernel(x, ln1_w, ln2_w, wq, wk, wv, wo, gate_w, w_fc, w_proj):
    global _cached_nc
    x = np.asarray(x, np.float32)
    B = x.shape[0]
    assert x.shape == (B, T, C) and B == 8
    weights = _prep_inputs(x, ln1_w, ln2_w, wq, wk, wv, wo, gate_w, w_fc, w_proj)
    in_maps = [dict(weights, x=np.ascontiguousarray(x[b])) for b in range(B)]
    if _cached_nc is None:
        _cached_nc = build_kernel()
    res = None
    last_err = None
    for attempt in range(3):
        try:
            res = bass_utils.run_bass_kernel_spmd(_cached_nc, in_maps,
                                                  core_ids=list(range(8)))
            break
        except Exception as err:  # transient NRT device errors: retry
            last_err = err
    if res is None:
        raise last_err
    y = np.stack([res.results[b]["y"] for b in range(B)])
    router = np.concatenate([res.results[b]["router"] for b in range(B)], axis=0)
    return y, router


# revision 18
# speedup vs baseline: 1.0086x; 1.0086x over previous
"""Trainium2 Bass kernel for nn_Block_47004122087942 (attention + dense top-2 MoE).

Strategy: pure data-parallel over batch (B=8 -> 8 NeuronCores, zero collectives).
Per core: one batch element [T=1024, C=512].
Compute dtype: float32r (fp32 storage, TF32-like matmul @ full PE rate) everywhere
except the gate-logit path, which runs in exact fp32 to reproduce the reference's
discrete top-2 expert selection.

Layout convention: "T" suffix = transposed [channels-on-partitions, tokens-on-free].
matmul(out[M,N], lhsT[K,M], rhs[K,N]) computes lhsT.T @ rhs with K = partitions.
"""
import numpy as np

import concourse.bass as bass
import concourse.tile as tile
from concourse import bacc, mybir
from concourse import bass_utils

P = 128
T = 1024          # tokens per core
C = 512           # channels
H = 8             # heads
D = 64            # head dim
E = 4             # experts
F = 2048          # ffn dim
NT = T // P       # 8 token tiles
NC = C // P       # 4 channel chunks
NF = F // P       # 16 ffn tiles
TCH = 512         # matmul free-dim chunk of tokens
NTCH = T // TCH   # 2
EPS = 1e-5

f32 = mybir.dt.float32
f32r = mybir.dt.float32r
bf16 = mybir.dt.bfloat16


def build_kernel():
    nc = bacc.Bacc("TRN2", target_bir_lowering=False, debug=False, num_devices=8)

    # ---- DRAM parameters (per-core shard views) ----
    x_ext = nc.dram_tensor("x", [T, C], f32, kind="ExternalInput")
    wqT_ext = nc.dram_tensor("wqT", [NC, P, C], f32, kind="ExternalInput")
    wkT_ext = nc.dram_tensor("wkT", [NC, P, C], f32, kind="ExternalInput")
    wvT_ext = nc.dram_tensor("wvT", [NC, P, C], f32, kind="ExternalInput")
    woT_ext = nc.dram_tensor("woT", [NC, P, C], f32, kind="ExternalInput")
    gateBC_ext = nc.dram_tensor("gateBC", [E, P, C], f32, kind="ExternalInput")
    wfcT_ext = nc.dram_tensor("wfcT", [E, NC, P, F], f32, kind="ExternalInput")
    wprojT_ext = nc.dram_tensor("wprojT", [E, NF, P, C], f32, kind="ExternalInput")
    maskT_ext = nc.dram_tensor("maskT", [4, P, TCH], f32, kind="ExternalInput")
    iden_ext = nc.dram_tensor("iden", [P, P], f32, kind="ExternalInput")
    y_ext = nc.dram_tensor("y", [T, C], f32, kind="ExternalOutput")
    router_ext = nc.dram_tensor("router", [T, E], f32, kind="ExternalOutput")

    rden_dram = [nc.dram_tensor(f"rden_scratch{kk}", [2, T], f32, kind="Internal")
                 for kk in range(NC)]

    with tile.TileContext(nc) as tc:
        from contextlib import ExitStack
        with ExitStack() as ctx:
            const = ctx.enter_context(tc.tile_pool(name="const", bufs=1))
            persist = ctx.enter_context(tc.tile_pool(name="persist", bufs=1))
            small = ctx.enter_context(tc.tile_pool(name="small", bufs=4))
            ntmp = ctx.enter_context(tc.tile_pool(name="ntmp", bufs=2))
            ps_mm = ctx.enter_context(tc.tile_pool(name="ps_mm", bufs=2, space="PSUM"))
            ps_tp = ctx.enter_context(tc.tile_pool(name="ps_tp", bufs=2, space="PSUM"))

            # ---- constants ----
            iden = const.tile([P, P], f32, name="iden")
            nc.sync.dma_start(out=iden[:], in_=iden_ext.ap())
            eps_t = const.tile([P, 1], f32, name="eps")
            nc.vector.memset(eps_t[:], EPS)
            ones8 = const.tile([P, H], f32, name="ones8")
            nc.vector.memset(ones8[:], 1.0)
            iden_r = const.tile([P, P], f32r, name="idenr")
            nc.gpsimd.dma_start(out=iden_r[:], in_=iden_ext.ap())

            # persistent activations
            x_sb = [persist.tile([P, C], f32, name=f"x{m}") for m in range(NT)]
            attT = [persist.tile([P, T], f32r, name=f"attT{k}") for k in range(NC)]
            xn2T_r = [persist.tile([P, T], bf16, name=f"xn2Tr{k}") for k in range(NC)]
            router_sb = [persist.tile([P, E], f32, name=f"rt{m}") for m in range(NT)]

            for m in range(NT):
                nc.sync.dma_start(out=x_sb[m][:], in_=x_ext.ap()[m * P:(m + 1) * P, :])

            # O-proj / gate weights loaded early so stage D can start the moment
            # attT is normalized (no SBUF-address-reuse dependency on the
            # attention scope close)
            woT_sb = []
            for kk in range(NC):
                w = persist.tile([P, C], f32r, name=f"wo{kk}")
                nc.gpsimd.dma_start(out=w[:], in_=woT_ext.ap()[kk])
                woT_sb.append(w)

            def rms_scale(xin, tag):
                """returns [P,1] f32 tile = 1/sqrt(mean(xin^2)+eps)"""
                sq = ntmp.tile([P, C], f32, name="sq")
                nc.vector.tensor_mul(out=sq[:], in0=xin, in1=xin)
                ssq = small.tile([P, 1], f32, name="ssq")
                nc.vector.tensor_reduce(out=ssq[:], in_=sq[:],
                                        axis=mybir.AxisListType.X, op=mybir.AluOpType.add)
                rms = small.tile([P, 1], f32, name="rms")
                nc.scalar.activation(out=rms[:], in_=ssq[:],
                                     func=mybir.ActivationFunctionType.Sqrt,
                                     bias=eps_t[:], scale=1.0 / C)
                rsc = small.tile([P, 1], f32, name="rsc")
                nc.vector.reciprocal(out=rsc[:], in_=rms[:])
                return rsc

            # =========================================================
            # attention scope
            # =========================================================
            with ExitStack() as actx:
                apool = actx.enter_context(tc.tile_pool(name="apool", bufs=1))
                expp = actx.enter_context(tc.tile_pool(name="expp", bufs=3))
                ps_sc = actx.enter_context(tc.tile_pool(name="ps_sc", bufs=2, space="PSUM"))
                ps_pv = actx.enter_context(tc.tile_pool(name="ps_pv", bufs=2, space="PSUM"))

                maskT = []
                for t in range(4):
                    mk = apool.tile([P, TCH], f32r, name=f"mask{t}")
                    nc.gpsimd.dma_start(out=mk[:], in_=maskT_ext.ap()[t])
                    maskT.append(mk)

                # ---- stage A: rms_norm1 + transpose -> xnT (f32r) ----
                bctx = ExitStack()
                bpool = bctx.enter_context(tc.tile_pool(name="bpool", bufs=1))
                xnT = [bpool.tile([P, T], f32r, name=f"xnT{k}") for k in range(NC)]
                for m in range(NT):
                    rsc = rms_scale(x_sb[m][:], f"n1_{m}")
                    xn = ntmp.tile([P, C], f32, name="xn")
                    nc.vector.tensor_scalar_mul(out=xn[:], in0=x_sb[m][:], scalar1=rsc[:])
                    for kk in range(NC):
                        tp = ps_tp.tile([P, P], f32, name="tp")
                        nc.tensor.transpose(tp[:], xn[:, kk * P:(kk + 1) * P], iden[:])
                        nc.vector.tensor_copy(out=xnT[kk][:, m * P:(m + 1) * P], in_=tp[:])

                # ---- stage B: QKV projections ----
                wqT_sb, wkT_sb, wvT_sb = [], [], []
                for kk in range(NC):
                    for lst, ext, nm in ((wqT_sb, wqT_ext, "wq"), (wkT_sb, wkT_ext, "wk"),
                                         (wvT_sb, wvT_ext, "wv")):
                        w = bpool.tile([P, C], f32r, name=f"{nm}{kk}")
                        nc.gpsimd.dma_start(out=w[:], in_=ext.ap()[kk])
                        lst.append(w)

                # q stored per-head, zero-padded to K=128 so the scores matmul
                # can contract over the full 128 partitions (the other head's k
                # rows meet zeros). k stored packed (2 heads per tile).
                zeros512 = apool.tile([P, TCH], f32, name="zeros512")
                nc.vector.memset(zeros512[:], 0.0)
                qz = [apool.tile([P, T], f32r, name=f"qz{h}") for h in range(H)]
                for h in range(H):
                    po = D * (h % 2)
                    zo = D - po  # the half that must be zeroed: 64 if h even else 0
                    for tch in range(NTCH):
                        nc.vector.tensor_copy(
                            out=qz[h][zo:zo + D, tch * TCH:(tch + 1) * TCH],
                            in_=zeros512[0:D, :])
                kT = [apool.tile([P, T], f32r, name=f"kT{o}") for o in range(NC)]
                for o4 in range(NC):
                    for tch in range(NTCH):
                        # K projection (packed)
                        pst = ps_mm.tile([P, TCH], f32, name="mmps")
                        for kk in range(NC):
                            nc.tensor.matmul(
                                pst[:],
                                lhsT=wkT_sb[kk][:, o4 * P:(o4 + 1) * P],
                                rhs=xnT[kk][:, tch * TCH:(tch + 1) * TCH],
                                start=(kk == 0), stop=(kk == NC - 1))
                        nc.vector.tensor_copy(
                            out=kT[o4][:, tch * TCH:(tch + 1) * TCH], in_=pst[:])
                        # Q projection (split per head, placed at matching offset)
                        pst = ps_mm.tile([P, TCH], f32, name="mmps")
                        for kk in range(NC):
                            nc.tensor.matmul(
                                pst[:],
                                lhsT=wqT_sb[kk][:, o4 * P:(o4 + 1) * P],
                                rhs=xnT[kk][:, tch * TCH:(tch + 1) * TCH],
                                start=(kk == 0), stop=(kk == NC - 1))
                        nc.vector.tensor_copy(
                            out=qz[2 * o4][0:D, tch * TCH:(tch + 1) * TCH],
                            in_=pst[0:D, :])
                        nc.vector.tensor_copy(
                            out=qz[2 * o4 + 1][D:P, tch * TCH:(tch + 1) * TCH],
                            in_=pst[D:P, :])

                # V in untransposed layout, interleaved per-head with ones column,
                # zero-padded to a full 128-wide stationary (M=128 matmuls are
                # much faster than M=65 on the fp32r path)
                VW = 128
                zrow = apool.tile([P, VW - D - 1], f32, name="zrow")
                nc.vector.memset(zrow[:], 0.0)
                v_aug = [apool.tile([P, H, VW], f32r, name=f"vaug{m}") for m in range(NT)]
                for m in range(NT):
                    pst = ps_mm.tile([P, C], f32, name="mmps")
                    for kk in range(NC):
                        nc.tensor.matmul(
                            pst[:],
                            lhsT=xnT[kk][:, m * P:(m + 1) * P],
                            rhs=wvT_sb[kk][:],
                            start=(kk == 0), stop=(kk == NC - 1))
                    nc.vector.tensor_copy(
                        out=v_aug[m][:, :, 0:D],
                        in_=pst[:].rearrange("p (h d) -> p h d", h=H))
                    nc.vector.tensor_copy(
                        out=v_aug[m][:, :, D:D + 1].rearrange("p h one -> p (h one)"),
                        in_=ones8[:])
                    for h in range(H):
                        nc.vector.tensor_copy(out=v_aug[m][:, h, D + 1:VW], in_=zrow[:])

                bctx.close()  # free xnT + QKV weights (40KB/partition)
                npool = actx.enter_context(tc.tile_pool(name="npool", bufs=1))

                # ---- stage C: attention, pair-major with pipelined PE emission ----
                for kk2 in range(NC):
                    for hh in range(2):
                        h = 2 * kk2 + hh
                        h2 = h // 2
                        po = D * (h % 2)
                        for ic in range(NTCH):
                            jmax = 4 * (ic + 1)
                            pv = ps_pv.tile([P, TCH], f32, name="pvps")
                            exs = []
                            pending_pv = []
                            for jj in range(jmax):
                                diag = jj >= 4 * ic
                                sc = ps_sc.tile([P, TCH], f32, name="scps")
                                nc.tensor.matmul(
                                    sc[:],
                                    lhsT=kT[h2][:, jj * P:(jj + 1) * P],
                                    rhs=qz[h][:, ic * TCH:(ic + 1) * TCH],
                                    start=True, stop=not diag)
                                if diag:
                                    # accumulate -1e30 additive causal mask on PE
                                    nc.tensor.matmul(
                                        sc[:],
                                        lhsT=iden_r[:],
                                        rhs=maskT[jj - 4 * ic][:],
                                        start=False, stop=True)
                                ex = expp.tile([P, TCH], f32r, name="expT")
                                nc.scalar.activation(out=ex[:], in_=sc[:],
                                                     func=mybir.ActivationFunctionType.Exp)
                                pending_pv.append((jj, ex))
                                # emit pv for the PREVIOUS jj so PE has a score
                                # matmul in flight while ACT computes this exp
                                if len(pending_pv) > 1:
                                    pjj, pex = pending_pv.pop(0)
                                    nc.tensor.matmul(
                                        pv[:, :],
                                        lhsT=v_aug[pjj][:, h, :],
                                        rhs=pex[:],
                                        start=(pjj == 0), stop=False)
                            pjj, pex = pending_pv.pop(0)
                            nc.tensor.matmul(
                                pv[:, :],
                                lhsT=v_aug[pjj][:, h, :],
                                rhs=pex[:],
                                start=(pjj == 0), stop=True)
                            # evict raw attention (f32r) + denominator row
                            nc.vector.tensor_copy(
                                out=attT[h2][po:po + D, ic * TCH:(ic + 1) * TCH],
                                in_=pv[0:D, :])
                            den_t = small.tile([1, TCH], f32, name="den")
                            nc.scalar.copy(out=den_t[:], in_=pv[D:D + 1, :])
                            nc.sync.dma_start(
                                out=rden_dram[kk2].ap()[hh:hh + 1, ic * TCH:(ic + 1) * TCH],
                                in_=den_t[:])
                    # normalize this kk pair immediately (overlaps later heads)
                    bc = npool.tile([P, T], f32, name="bcast")
                    for hh in range(2):
                        srcap = rden_dram[kk2].ap()[hh:hh + 1, :]
                        bsrc = bass.AP(tensor=srcap.tensor, offset=srcap.offset,
                                       ap=[[0, D]] + srcap.ap[1:])
                        nc.sync.dma_start(out=bc[hh * D:(hh + 1) * D, :], in_=bsrc)
                    bc2 = npool.tile([P, T], f32, name="bcrec")
                    scr = npool.tile([P, T], f32, name="bcscr")
                    nc.vector.reciprocal_approx_accurate(out=bc2[:], in_=bc[:], scratch=scr[:])
                    nc.vector.tensor_tensor(out=attT[kk2][:], in0=attT[kk2][:],
                                            in1=bc2[:], op=mybir.AluOpType.mult)

                # =========================================================
                # stage D (inside attention scope: no release barrier between
                # the last PV and the O-projection)
                # =========================================================
                gate_bc = []
                for e in range(E):
                    gb = npool.tile([P, C], f32, name=f"gbc{e}")
                    nc.sync.dma_start(out=gb[:], in_=gateBC_ext.ap()[e])
                    gate_bc.append(gb)

                for m in range(NT):
                    pat = ps_mm.tile([P, C], f32, name="mmps")
                    for kk in range(NC):
                        nc.tensor.matmul(
                            pat[:],
                            lhsT=attT[kk][:, m * P:(m + 1) * P],
                            rhs=woT_sb[kk][:],
                            start=(kk == 0), stop=(kk == NC - 1))
                    # x1 = x + att_out (in place into x_sb)
                    nc.vector.tensor_tensor(out=x_sb[m][:], in0=x_sb[m][:], in1=pat[:],
                                            op=mybir.AluOpType.add)
                    rsc = rms_scale(x_sb[m][:], f"n2_{m}")
                    xn2 = ntmp.tile([P, C], f32, name="xn2")
                    nc.vector.tensor_scalar_mul(out=xn2[:], in0=x_sb[m][:], scalar1=rsc[:])
                    for kk in range(NC):
                        tp = ps_tp.tile([P, P], f32, name="tp")
                        nc.tensor.transpose(tp[:], xn2[:, kk * P:(kk + 1) * P], iden[:])
                        nc.vector.tensor_copy(out=xn2T_r[kk][:, m * P:(m + 1) * P], in_=tp[:])

                    # gate logits in exact fp32 on DVE: fused mult + free-axis
                    # reduce against pre-broadcast gate rows
                    glog = small.tile([P, E], f32, name="glog")
                    for e in range(E):
                        gsc = ntmp.tile([P, C], f32, name="sq")
                        nc.vector.scalar_tensor_tensor(
                            out=gsc[:], in0=xn2[:], scalar=1.0, in1=gate_bc[e][:],
                            op0=mybir.AluOpType.mult, op1=mybir.AluOpType.mult,
                            accum_out=glog[:, e:e + 1])
                    g = glog[:]
                    m1 = small.tile([P, 1], f32, name="m1")
                    nc.vector.tensor_reduce(out=m1[:], in_=g, axis=mybir.AxisListType.X,
                                            op=mybir.AluOpType.max)
                    m1n = small.tile([P, 1], f32, name="m1n")
                    nc.vector.tensor_scalar_mul(out=m1n[:], in0=m1[:], scalar1=-1.0)
                    ex = small.tile([P, E], f32, name="gex")
                    nc.scalar.activation(out=ex[:], in_=g,
                                         func=mybir.ActivationFunctionType.Exp,
                                         bias=m1n[:], scale=1.0)
                    eq = small.tile([P, E], f32, name="geq")
                    nc.vector.tensor_scalar(out=eq[:], in0=g, scalar1=m1[:], scalar2=None,
                                            op0=mybir.AluOpType.is_ge)
                    gm = small.tile([P, E], f32, name="ggm")
                    nc.vector.scalar_tensor_tensor(out=gm[:], in0=eq[:], scalar=-1e30,
                                                   in1=g, op0=mybir.AluOpType.mult,
                                                   op1=mybir.AluOpType.add)
                    m2 = small.tile([P, 1], f32, name="m2")
                    nc.vector.tensor_reduce(out=m2[:], in_=gm[:], axis=mybir.AxisListType.X,
                                            op=mybir.AluOpType.max)
                    keep = small.tile([P, E], f32, name="gkeep")
                    nc.vector.tensor_scalar(out=keep[:], in0=g, scalar1=m2[:], scalar2=None,
                                            op0=mybir.AluOpType.is_ge)
                    wun = small.tile([P, E], f32, name="gwun")
                    nc.vector.tensor_tensor(out=wun[:], in0=ex[:], in1=keep[:],
                                            op=mybir.AluOpType.mult)
                    s = small.tile([P, 1], f32, name="gsum")
                    nc.vector.tensor_reduce(out=s[:], in_=wun[:], axis=mybir.AxisListType.X,
                                            op=mybir.AluOpType.add)
                    rs = small.tile([P, 1], f32, name="grs")
                    nc.vector.reciprocal(out=rs[:], in_=s[:])
                    nc.vector.tensor_scalar_mul(out=router_sb[m][:], in0=wun[:], scalar1=rs[:])
                    nc.sync.dma_start(out=router_ext.ap()[m * P:(m + 1) * P, :],
                                      in_=router_sb[m][:])
            # attention + stage D scope closed

            # =========================================================
            # stage E scope: MoE pools
            # =========================================================
            with ExitStack() as dctx:
                wfcp = dctx.enter_context(tc.tile_pool(name="wfcp", bufs=2))
                wprp = dctx.enter_context(tc.tile_pool(name="wprp", bufs=1))
                hp = dctx.enter_context(tc.tile_pool(name="hp", bufs=3))
                ps_o = dctx.enter_context(tc.tile_pool(name="ps_o", bufs=4, space="PSUM"))

                def load_expert(e):
                    fc, pr = [], []
                    for kk in range(NC):
                        w = wfcp.tile([P, F], bf16, name=f"wfc{kk}")
                        nc.gpsimd.dma_start(out=w[:], in_=wfcT_ext.ap()[e, kk])
                        fc.append(w)
                    for k in range(NF):
                        w = wprp.tile([P, C], bf16, name=f"wpr{k}")
                        nc.gpsimd.dma_start(out=w[:], in_=wprojT_ext.ap()[e, k])
                        pr.append(w)
                    return fc, pr

                next_w = load_expert(0)

                # =========================================================
                for e in range(E):
                    wfc_sb, wpr_sb = next_w
                    if e + 1 < E:
                        next_w = load_expert(e + 1)
                    for tch in range(NTCH):
                        po_tiles = [ps_o.tile([P, C], f32, name="moeo")
                                    for i in range(4)]
                        pending = []
                        for k in range(NF):
                            ph = ps_mm.tile([P, TCH], f32, name="mmps")
                            for kk in range(NC):
                                nc.tensor.matmul(
                                    ph[:],
                                    lhsT=wfc_sb[kk][:, k * P:(k + 1) * P],
                                    rhs=xn2T_r[kk][:, tch * TCH:(tch + 1) * TCH],
                                    start=(kk == 0), stop=(kk == NC - 1))
                            hT = hp.tile([P, TCH], bf16, name="hT")
                            nc.scalar.activation(out=hT[:], in_=ph[:],
                                                 func=mybir.ActivationFunctionType.Gelu)
                            pending.append((k, hT))
                            if len(pending) > 1:
                                pk, phT = pending.pop(0)
                                for i in range(4):
                                    nc.tensor.matmul(
                                        po_tiles[i][:],
                                        lhsT=phT[:, i * P:(i + 1) * P],
                                        rhs=wpr_sb[pk][:],
                                        start=(pk == 0), stop=False)
                        pk, phT = pending.pop(0)
                        for i in range(4):
                            nc.tensor.matmul(
                                po_tiles[i][:],
                                lhsT=phT[:, i * P:(i + 1) * P],
                                rhs=wpr_sb[pk][:],
                                start=(pk == 0), stop=True)
                        for i in range(4):
                            m = 4 * tch + i
                            nc.vector.scalar_tensor_tensor(
                                out=x_sb[m][:], in0=po_tiles[i][:],
                                scalar=router_sb[m][:, e:e + 1], in1=x_sb[m][:],
                                op0=mybir.AluOpType.mult, op1=mybir.AluOpType.add)

            for m in range(NT):
                nc.sync.dma_start(out=y_ext.ap()[m * P:(m + 1) * P, :], in_=x_sb[m][:])

    nc.compile()
    return nc


def _prep_inputs(x, ln1_w, ln2_w, wq, wk, wv, wo, gate_w, w_fc, w_proj):
    """Host-side weight preprocessing: fold rms-norm gains + score scale,
    pre-transpose for TensorE (contraction dim on partitions)."""
    f = np.float32
    ln1 = np.asarray(ln1_w, f)
    ln2 = np.asarray(ln2_w, f)
    wqT = np.ascontiguousarray((np.asarray(wq, f) * ln1[None, :]).T).reshape(NC, P, C)
    wkT = np.ascontiguousarray((np.asarray(wk, f) * ln1[None, :]).T / np.sqrt(np.float32(D))).reshape(NC, P, C)
    wvT = np.ascontiguousarray((np.asarray(wv, f) * ln1[None, :]).T).reshape(NC, P, C)
    woT = np.ascontiguousarray(np.asarray(wo, f).T).reshape(NC, P, C)
    gateBC = np.ascontiguousarray(
        np.repeat((np.asarray(gate_w, f) * ln2[None, :])[:, None, :], P, axis=1))
    wfcT = np.stack([
        np.ascontiguousarray((np.asarray(w_fc, f)[e] * ln2[None, :]).T).reshape(NC, P, F)
        for e in range(E)])
    wprojT = np.stack([
        np.ascontiguousarray(np.asarray(w_proj, f)[e].T).reshape(NF, P, C)
        for e in range(E)])
    # additive causal masks in transposed-scores layout: tile covers j in
    # [128t, 128t+128), i in [0, 512) relative; keep (0.0) iff
    # j_local <= i_local - 128t else -1e30 (exp -> 0)
    maskT = np.zeros((4, P, TCH), f)
    jl = np.arange(P)[:, None]
    il = np.arange(TCH)[None, :]
    for t in range(4):
        maskT[t] = np.where(jl <= il - P * t, f(0.0), f(-1e30))
    iden = np.eye(P, dtype=f)
    return dict(wqT=wqT, wkT=wkT, wvT=wvT, woT=woT, gateBC=gateBC,
                wfcT=wfcT, wprojT=wprojT, maskT=maskT, iden=iden)


_cached_nc = None


def kernel(x, ln1_w, ln2_w, wq, wk, wv, wo, gate_w, w_fc, w_proj):
    global _cached_nc
    x = np.asarray(x, np.float32)
    B = x.shape[0]
    assert x.shape == (B, T, C) and B == 8
    weights = _prep_inputs(x, ln1_w, ln2_w, wq, wk, wv, wo, gate_w, w_fc, w_proj)
    in_maps = [dict(weights, x=np.ascontiguousarray(x[b])) for b in range(B)]
    if _cached_nc is None:
        _cached_nc = build_kernel()
    res = None
    last_err = None
    for attempt in range(3):
        try:
            res = bass_utils.run_bass_kernel_spmd(_cached_nc, in_maps,
                                                  core_ids=list(range(8)))
            break
        except Exception as err:  # transient NRT device errors: retry
            last_err = err
    if res is None:
        raise last_err
    y = np.stack([res.results[b]["y"] for b in range(B)])
    router = np.concatenate([res.results[b]["router"] for b in range(B)], axis=0)
    return y, router


# revision 19
# speedup vs baseline: 1.0333x; 1.0245x over previous
"""Trainium2 Bass kernel for nn_Block_47004122087942 (attention + dense top-2 MoE).

Strategy: pure data-parallel over batch (B=8 -> 8 NeuronCores, zero collectives).
Per core: one batch element [T=1024, C=512].
Compute dtype: float32r (fp32 storage, TF32-like matmul @ full PE rate) everywhere
except the gate-logit path, which runs in exact fp32 to reproduce the reference's
discrete top-2 expert selection.

Layout convention: "T" suffix = transposed [channels-on-partitions, tokens-on-free].
matmul(out[M,N], lhsT[K,M], rhs[K,N]) computes lhsT.T @ rhs with K = partitions.
"""
import numpy as np

import concourse.bass as bass
import concourse.tile as tile
from concourse import bacc, mybir
from concourse import bass_utils

P = 128
T = 1024          # tokens per core
C = 512           # channels
H = 8             # heads
D = 64            # head dim
E = 4             # experts
F = 2048          # ffn dim
NT = T // P       # 8 token tiles
NC = C // P       # 4 channel chunks
NF = F // P       # 16 ffn tiles
TCH = 512         # matmul free-dim chunk of tokens
NTCH = T // TCH   # 2
EPS = 1e-5

f32 = mybir.dt.float32
f32r = mybir.dt.float32r
bf16 = mybir.dt.bfloat16


def build_kernel():
    nc = bacc.Bacc("TRN2", target_bir_lowering=False, debug=False, num_devices=8)

    # ---- DRAM parameters (per-core shard views) ----
    x_ext = nc.dram_tensor("x", [T, C], f32, kind="ExternalInput")
    wqT_ext = nc.dram_tensor("wqT", [NC, P, C], f32, kind="ExternalInput")
    wkT_ext = nc.dram_tensor("wkT", [NC, P, C], f32, kind="ExternalInput")
    wvT_ext = nc.dram_tensor("wvT", [NC, P, C], f32, kind="ExternalInput")
    woT_ext = nc.dram_tensor("woT", [NC, P, C], f32, kind="ExternalInput")
    gateBC_ext = nc.dram_tensor("gateBC", [E, P, C], f32, kind="ExternalInput")
    wfcT_ext = nc.dram_tensor("wfcT", [E, NC, P, F], f32, kind="ExternalInput")
    wprojT_ext = nc.dram_tensor("wprojT", [E, NF, P, C], f32, kind="ExternalInput")
    maskT_ext = nc.dram_tensor("maskT", [4, P, TCH], f32, kind="ExternalInput")
    iden_ext = nc.dram_tensor("iden", [P, P], f32, kind="ExternalInput")
    y_ext = nc.dram_tensor("y", [T, C], f32, kind="ExternalOutput")
    router_ext = nc.dram_tensor("router", [T, E], f32, kind="ExternalOutput")

    rden_dram = [nc.dram_tensor(f"rden_scratch{kk}", [2, T], f32, kind="Internal")
                 for kk in range(NC)]

    with tile.TileContext(nc) as tc:
        from contextlib import ExitStack
        with ExitStack() as ctx:
            const = ctx.enter_context(tc.tile_pool(name="const", bufs=1))
            persist = ctx.enter_context(tc.tile_pool(name="persist", bufs=1))
            small = ctx.enter_context(tc.tile_pool(name="small", bufs=4))
            ntmp = ctx.enter_context(tc.tile_pool(name="ntmp", bufs=2))
            ps_mm = ctx.enter_context(tc.tile_pool(name="ps_mm", bufs=2, space="PSUM"))
            ps_tp = ctx.enter_context(tc.tile_pool(name="ps_tp", bufs=2, space="PSUM"))

            # ---- constants ----
            iden = const.tile([P, P], f32, name="iden")
            nc.sync.dma_start(out=iden[:], in_=iden_ext.ap())
            eps_t = const.tile([P, 1], f32, name="eps")
            nc.vector.memset(eps_t[:], EPS)
            ones8 = const.tile([P, H], f32, name="ones8")
            nc.vector.memset(ones8[:], 1.0)
            iden_r = const.tile([P, P], f32r, name="idenr")
            nc.gpsimd.dma_start(out=iden_r[:], in_=iden_ext.ap())

            # persistent activations
            x_sb = [persist.tile([P, C], f32, name=f"x{m}") for m in range(NT)]
            attT = [persist.tile([P, T], f32r, name=f"attT{k}") for k in range(NC)]
            xn2T_r = [persist.tile([P, T], bf16, name=f"xn2Tr{k}") for k in range(NC)]
            router_sb = [persist.tile([P, E], f32, name=f"rt{m}") for m in range(NT)]

            for m in range(NT):
                nc.sync.dma_start(out=x_sb[m][:], in_=x_ext.ap()[m * P:(m + 1) * P, :])

            # O-proj / gate weights loaded early so stage D can start the moment
            # attT is normalized (no SBUF-address-reuse dependency on the
            # attention scope close)
            woT_sb = []
            for kk in range(NC):
                w = persist.tile([P, C], f32r, name=f"wo{kk}")
                nc.gpsimd.dma_start(out=w[:], in_=woT_ext.ap()[kk])
                woT_sb.append(w)

            def rms_scale(xin, tag):
                """returns [P,1] f32 tile = 1/sqrt(mean(xin^2)+eps)"""
                sq = ntmp.tile([P, C], f32, name="sq")
                nc.vector.tensor_mul(out=sq[:], in0=xin, in1=xin)
                ssq = small.tile([P, 1], f32, name="ssq")
                nc.vector.tensor_reduce(out=ssq[:], in_=sq[:],
                                        axis=mybir.AxisListType.X, op=mybir.AluOpType.add)
                rms = small.tile([P, 1], f32, name="rms")
                nc.scalar.activation(out=rms[:], in_=ssq[:],
                                     func=mybir.ActivationFunctionType.Sqrt,
                                     bias=eps_t[:], scale=1.0 / C)
                rsc = small.tile([P, 1], f32, name="rsc")
                nc.vector.reciprocal(out=rsc[:], in_=rms[:])
                return rsc

            # =========================================================
            # attention scope
            # =========================================================
            with ExitStack() as actx:
                apool = actx.enter_context(tc.tile_pool(name="apool", bufs=1))
                expp = actx.enter_context(tc.tile_pool(name="expp", bufs=3))
                ps_sc = actx.enter_context(tc.tile_pool(name="ps_sc", bufs=2, space="PSUM"))
                ps_pv = actx.enter_context(tc.tile_pool(name="ps_pv", bufs=2, space="PSUM"))

                maskT = []
                for t in range(4):
                    mk = apool.tile([P, TCH], f32r, name=f"mask{t}")
                    nc.gpsimd.dma_start(out=mk[:], in_=maskT_ext.ap()[t])
                    maskT.append(mk)

                # ---- stage A: rms_norm1 + transpose -> xnT (f32r) ----
                bctx = ExitStack()
                bpool = bctx.enter_context(tc.tile_pool(name="bpool", bufs=1))
                xnT = [bpool.tile([P, T], f32r, name=f"xnT{k}") for k in range(NC)]
                for m in range(NT):
                    rsc = rms_scale(x_sb[m][:], f"n1_{m}")
                    xn = ntmp.tile([P, C], f32, name="xn")
                    nc.vector.tensor_scalar_mul(out=xn[:], in0=x_sb[m][:], scalar1=rsc[:])
                    for kk in range(NC):
                        tp = ps_tp.tile([P, P], f32, name="tp")
                        nc.tensor.transpose(tp[:], xn[:, kk * P:(kk + 1) * P], iden[:])
                        nc.scalar.copy(out=xnT[kk][:, m * P:(m + 1) * P], in_=tp[:])

                # ---- stage B: QKV projections ----
                wqT_sb, wkT_sb, wvT_sb = [], [], []
                for kk in range(NC):
                    for lst, ext, nm in ((wqT_sb, wqT_ext, "wq"), (wkT_sb, wkT_ext, "wk"),
                                         (wvT_sb, wvT_ext, "wv")):
                        w = bpool.tile([P, C], f32r, name=f"{nm}{kk}")
                        nc.gpsimd.dma_start(out=w[:], in_=ext.ap()[kk])
                        lst.append(w)

                # q stored per-head, zero-padded to K=128 so the scores matmul
                # can contract over the full 128 partitions (the other head's k
                # rows meet zeros). k stored packed (2 heads per tile).
                zeros512 = apool.tile([P, TCH], f32, name="zeros512")
                nc.vector.memset(zeros512[:], 0.0)
                qz = [apool.tile([P, T], f32r, name=f"qz{h}") for h in range(H)]
                for h in range(H):
                    po = D * (h % 2)
                    zo = D - po  # the half that must be zeroed: 64 if h even else 0
                    for tch in range(NTCH):
                        nc.vector.tensor_copy(
                            out=qz[h][zo:zo + D, tch * TCH:(tch + 1) * TCH],
                            in_=zeros512[0:D, :])
                kT = [apool.tile([P, T], f32r, name=f"kT{o}") for o in range(NC)]
                for o4 in range(NC):
                    for tch in range(NTCH):
                        # K projection (packed)
                        pst = ps_mm.tile([P, TCH], f32, name="mmps")
                        for kk in range(NC):
                            nc.tensor.matmul(
                                pst[:],
                                lhsT=wkT_sb[kk][:, o4 * P:(o4 + 1) * P],
                                rhs=xnT[kk][:, tch * TCH:(tch + 1) * TCH],
                                start=(kk == 0), stop=(kk == NC - 1))
                        nc.scalar.copy(
                            out=kT[o4][:, tch * TCH:(tch + 1) * TCH], in_=pst[:])
                        # Q projection (split per head, placed at matching offset)
                        pst = ps_mm.tile([P, TCH], f32, name="mmps")
                        for kk in range(NC):
                            nc.tensor.matmul(
                                pst[:],
                                lhsT=wqT_sb[kk][:, o4 * P:(o4 + 1) * P],
                                rhs=xnT[kk][:, tch * TCH:(tch + 1) * TCH],
                                start=(kk == 0), stop=(kk == NC - 1))
                        nc.scalar.copy(
                            out=qz[2 * o4][0:D, tch * TCH:(tch + 1) * TCH],
                            in_=pst[0:D, :])
                        nc.scalar.copy(
                            out=qz[2 * o4 + 1][D:P, tch * TCH:(tch + 1) * TCH],
                            in_=pst[D:P, :])

                # V in untransposed layout, interleaved per-head with ones column,
                # zero-padded to a full 128-wide stationary (M=128 matmuls are
                # much faster than M=65 on the fp32r path)
                VW = 128
                zrow = apool.tile([P, VW - D - 1], f32, name="zrow")
                nc.vector.memset(zrow[:], 0.0)
                v_aug = [apool.tile([P, H, VW], f32r, name=f"vaug{m}") for m in range(NT)]
                for m in range(NT):
                    pst = ps_mm.tile([P, C], f32, name="mmps")
                    for kk in range(NC):
                        nc.tensor.matmul(
                            pst[:],
                            lhsT=xnT[kk][:, m * P:(m + 1) * P],
                            rhs=wvT_sb[kk][:],
                            start=(kk == 0), stop=(kk == NC - 1))
                    nc.scalar.copy(
                        out=v_aug[m][:, :, 0:D],
                        in_=pst[:].rearrange("p (h d) -> p h d", h=H))
                    nc.vector.tensor_copy(
                        out=v_aug[m][:, :, D:D + 1].rearrange("p h one -> p (h one)"),
                        in_=ones8[:])
                    for h in range(H):
                        nc.vector.tensor_copy(out=v_aug[m][:, h, D + 1:VW], in_=zrow[:])

                bctx.close()  # free xnT + QKV weights (40KB/partition)
                npool = actx.enter_context(tc.tile_pool(name="npool", bufs=1))

                # ---- stage C: attention, pair-major with pipelined PE emission ----
                for kk2 in range(NC):
                    for hh in range(2):
                        h = 2 * kk2 + hh
                        h2 = h // 2
                        po = D * (h % 2)
                        for ic in range(NTCH):
                            jmax = 4 * (ic + 1)
                            pv = ps_pv.tile([P, TCH], f32, name="pvps")
                            exs = []
                            pending_pv = []
                            for jj in range(jmax):
                                diag = jj >= 4 * ic
                                sc = ps_sc.tile([P, TCH], f32, name="scps")
                                nc.tensor.matmul(
                                    sc[:],
                                    lhsT=kT[h2][:, jj * P:(jj + 1) * P],
                                    rhs=qz[h][:, ic * TCH:(ic + 1) * TCH],
                                    start=True, stop=not diag)
                                if diag:
                                    # accumulate -1e30 additive causal mask on PE
                                    nc.tensor.matmul(
                                        sc[:],
                                        lhsT=iden_r[:],
                                        rhs=maskT[jj - 4 * ic][:],
                                        start=False, stop=True)
                                ex = expp.tile([P, TCH], f32r, name="expT")
                                nc.scalar.activation(out=ex[:], in_=sc[:],
                                                     func=mybir.ActivationFunctionType.Exp)
                                pending_pv.append((jj, ex))
                                # emit pv for the PREVIOUS jj so PE has a score
                                # matmul in flight while ACT computes this exp
                                if len(pending_pv) > 1:
                                    pjj, pex = pending_pv.pop(0)
                                    nc.tensor.matmul(
                                        pv[:, :],
                                        lhsT=v_aug[pjj][:, h, :],
                                        rhs=pex[:],
                                        start=(pjj == 0), stop=False)
                            pjj, pex = pending_pv.pop(0)
                            nc.tensor.matmul(
                                pv[:, :],
                                lhsT=v_aug[pjj][:, h, :],
                                rhs=pex[:],
                                start=(pjj == 0), stop=True)
                            # evict raw attention (f32r) + denominator row
                            nc.vector.tensor_copy(
                                out=attT[h2][po:po + D, ic * TCH:(ic + 1) * TCH],
                                in_=pv[0:D, :])
                            den_t = small.tile([1, TCH], f32, name="den")
                            nc.scalar.copy(out=den_t[:], in_=pv[D:D + 1, :])
                            nc.sync.dma_start(
                                out=rden_dram[kk2].ap()[hh:hh + 1, ic * TCH:(ic + 1) * TCH],
                                in_=den_t[:])
                    # normalize this kk pair immediately (overlaps later heads)
                    bc = npool.tile([P, T], f32, name="bcast")
                    for hh in range(2):
                        srcap = rden_dram[kk2].ap()[hh:hh + 1, :]
                        bsrc = bass.AP(tensor=srcap.tensor, offset=srcap.offset,
                                       ap=[[0, D]] + srcap.ap[1:])
                        nc.sync.dma_start(out=bc[hh * D:(hh + 1) * D, :], in_=bsrc)
                    bc2 = npool.tile([P, T], f32, name="bcrec")
                    scr = npool.tile([P, T], f32, name="bcscr")
                    nc.vector.reciprocal_approx_accurate(out=bc2[:], in_=bc[:], scratch=scr[:])
                    nc.vector.tensor_tensor(out=attT[kk2][:], in0=attT[kk2][:],
                                            in1=bc2[:], op=mybir.AluOpType.mult)

                # =========================================================
                # stage D (inside attention scope: no release barrier between
                # the last PV and the O-projection)
                # =========================================================
                gate_bc = []
                for e in range(E):
                    gb = npool.tile([P, C], f32, name=f"gbc{e}")
                    nc.sync.dma_start(out=gb[:], in_=gateBC_ext.ap()[e])
                    gate_bc.append(gb)

                for m in range(NT):
                    pat = ps_mm.tile([P, C], f32, name="mmps")
                    for kk in range(NC):
                        nc.tensor.matmul(
                            pat[:],
                            lhsT=attT[kk][:, m * P:(m + 1) * P],
                            rhs=woT_sb[kk][:],
                            start=(kk == 0), stop=(kk == NC - 1))
                    # x1 = x + att_out (in place into x_sb)
                    nc.vector.tensor_tensor(out=x_sb[m][:], in0=x_sb[m][:], in1=pat[:],
                                            op=mybir.AluOpType.add)
                    rsc = rms_scale(x_sb[m][:], f"n2_{m}")
                    xn2 = ntmp.tile([P, C], f32, name="xn2")
                    nc.vector.tensor_scalar_mul(out=xn2[:], in0=x_sb[m][:], scalar1=rsc[:])
                    for kk in range(NC):
                        tp = ps_tp.tile([P, P], f32, name="tp")
                        nc.tensor.transpose(tp[:], xn2[:, kk * P:(kk + 1) * P], iden[:])
                        nc.vector.tensor_copy(out=xn2T_r[kk][:, m * P:(m + 1) * P], in_=tp[:])

                    # gate logits in exact fp32 on DVE: fused mult + free-axis
                    # reduce against pre-broadcast gate rows
                    glog = small.tile([P, E], f32, name="glog")
                    for e in range(E):
                        gsc = ntmp.tile([P, C], f32, name="sq")
                        nc.vector.scalar_tensor_tensor(
                            out=gsc[:], in0=xn2[:], scalar=1.0, in1=gate_bc[e][:],
                            op0=mybir.AluOpType.mult, op1=mybir.AluOpType.mult,
                            accum_out=glog[:, e:e + 1])
                    g = glog[:]
                    m1 = small.tile([P, 1], f32, name="m1")
                    nc.vector.tensor_reduce(out=m1[:], in_=g, axis=mybir.AxisListType.X,
                                            op=mybir.AluOpType.max)
                    m1n = small.tile([P, 1], f32, name="m1n")
                    nc.vector.tensor_scalar_mul(out=m1n[:], in0=m1[:], scalar1=-1.0)
                    ex = small.tile([P, E], f32, name="gex")
                    nc.scalar.activation(out=ex[:], in_=g,
                                         func=mybir.ActivationFunctionType.Exp,
                                         bias=m1n[:], scale=1.0)
                    eq = small.tile([P, E], f32, name="geq")
                    nc.vector.tensor_scalar(out=eq[:], in0=g, scalar1=m1[:], scalar2=None,
                                            op0=mybir.AluOpType.is_ge)
                    gm = small.tile([P, E], f32, name="ggm")
                    nc.vector.scalar_tensor_tensor(out=gm[:], in0=eq[:], scalar=-1e30,
                                                   in1=g, op0=mybir.AluOpType.mult,
                                                   op1=mybir.AluOpType.add)
                    m2 = small.tile([P, 1], f32, name="m2")
                    nc.vector.tensor_reduce(out=m2[:], in_=gm[:], axis=mybir.AxisListType.X,
                                            op=mybir.AluOpType.max)
                    keep = small.tile([P, E], f32, name="gkeep")
                    nc.vector.tensor_scalar(out=keep[:], in0=g, scalar1=m2[:], scalar2=None,
                                            op0=mybir.AluOpType.is_ge)
                    wun = small.tile([P, E], f32, name="gwun")
                    nc.vector.tensor_tensor(out=wun[:], in0=ex[:], in1=keep[:],
                                            op=mybir.AluOpType.mult)
                    s = small.tile([P, 1], f32, name="gsum")
                    nc.vector.tensor_reduce(out=s[:], in_=wun[:], axis=mybir.AxisListType.X,
                                            op=mybir.AluOpType.add)
                    rs = small.tile([P, 1], f32, name="grs")
                    nc.vector.reciprocal(out=rs[:], in_=s[:])
                    nc.vector.tensor_scalar_mul(out=router_sb[m][:], in0=wun[:], scalar1=rs[:])
                    nc.sync.dma_start(out=router_ext.ap()[m * P:(m + 1) * P, :],
                                      in_=router_sb[m][:])
            # attention + stage D scope closed

            # =========================================================
            # stage E scope: MoE pools
            # =========================================================
            with ExitStack() as dctx:
                wfcp = dctx.enter_context(tc.tile_pool(name="wfcp", bufs=2))
                wprp = dctx.enter_context(tc.tile_pool(name="wprp", bufs=1))
                hp = dctx.enter_context(tc.tile_pool(name="hp", bufs=3))
                ps_o = dctx.enter_context(tc.tile_pool(name="ps_o", bufs=4, space="PSUM"))

                def load_expert(e):
                    fc, pr = [], []
                    for kk in range(NC):
                        w = wfcp.tile([P, F], bf16, name=f"wfc{kk}")
                        nc.gpsimd.dma_start(out=w[:], in_=wfcT_ext.ap()[e, kk])
                        fc.append(w)
                    for k in range(NF):
                        w = wprp.tile([P, C], bf16, name=f"wpr{k}")
                        nc.gpsimd.dma_start(out=w[:], in_=wprojT_ext.ap()[e, k])
                        pr.append(w)
                    return fc, pr

                next_w = load_expert(0)

                # =========================================================
                for e in range(E):
                    wfc_sb, wpr_sb = next_w
                    if e + 1 < E:
                        next_w = load_expert(e + 1)
                    for tch in range(NTCH):
                        po_tiles = [ps_o.tile([P, C], f32, name="moeo")
                                    for i in range(4)]
                        pending = []
                        for k in range(NF):
                            ph = ps_mm.tile([P, TCH], f32, name="mmps")
                            for kk in range(NC):
                                nc.tensor.matmul(
                                    ph[:],
                                    lhsT=wfc_sb[kk][:, k * P:(k + 1) * P],
                                    rhs=xn2T_r[kk][:, tch * TCH:(tch + 1) * TCH],
                                    start=(kk == 0), stop=(kk == NC - 1))
                            hT = hp.tile([P, TCH], bf16, name="hT")
                            nc.scalar.activation(out=hT[:], in_=ph[:],
                                                 func=mybir.ActivationFunctionType.Gelu)
                            pending.append((k, hT))
                            if len(pending) > 1:
                                pk, phT = pending.pop(0)
                                for i in range(4):
                                    nc.tensor.matmul(
                                        po_tiles[i][:],
                                        lhsT=phT[:, i * P:(i + 1) * P],
                                        rhs=wpr_sb[pk][:],
                                        start=(pk == 0), stop=False)
                        pk, phT = pending.pop(0)
                        for i in range(4):
                            nc.tensor.matmul(
                                po_tiles[i][:],
                                lhsT=phT[:, i * P:(i + 1) * P],
                                rhs=wpr_sb[pk][:],
                                start=(pk == 0), stop=True)
                        for i in range(4):
                            m = 4 * tch + i
                            nc.vector.scalar_tensor_tensor(
                                out=x_sb[m][:], in0=po_tiles[i][:],
                                scalar=router_sb[m][:, e:e + 1], in1=x_sb[m][:],
                                op0=mybir.AluOpType.mult, op1=mybir.AluOpType.add)

            for m in range(NT):
                nc.sync.dma_start(out=y_ext.ap()[m * P:(m + 1) * P, :], in_=x_sb[m][:])

    nc.compile()
    return nc


def _prep_inputs(x, ln1_w, ln2_w, wq, wk, wv, wo, gate_w, w_fc, w_proj):
    """Host-side weight preprocessing: fold rms-norm gains + score scale,
    pre-transpose for TensorE (contraction dim on partitions)."""
    f = np.float32
    ln1 = np.asarray(ln1_w, f)
    ln2 = np.asarray(ln2_w, f)
    wqT = np.ascontiguousarray((np.asarray(wq, f) * ln1[None, :]).T).reshape(NC, P, C)
    wkT = np.ascontiguousarray((np.asarray(wk, f) * ln1[None, :]).T / np.sqrt(np.float32(D))).reshape(NC, P, C)
    wvT = np.ascontiguousarray((np.asarray(wv, f) * ln1[None, :]).T).reshape(NC, P, C)
    woT = np.ascontiguousarray(np.asarray(wo, f).T).reshape(NC, P, C)
    gateBC = np.ascontiguousarray(
        np.repeat((np.asarray(gate_w, f) * ln2[None, :])[:, None, :], P, axis=1))
    wfcT = np.stack([
        np.ascontiguousarray((np.asarray(w_fc, f)[e] * ln2[None, :]).T).reshape(NC, P, F)
        for e in range(E)])
    wprojT = np.stack([
        np.ascontiguousarray(np.asarray(w_proj, f)[e].T).reshape(NF, P, C)
        for e in range(E)])
    # additive causal masks in transposed-scores layout: tile covers j in
    # [128t, 128t+128), i in [0, 512) relative; keep (0.0) iff
    # j_local <= i_local - 128t else -1e30 (exp -> 0)
    maskT = np.zeros((4, P, TCH), f)
    jl = np.arange(P)[:, None]
    il = np.arange(TCH)[None, :]
    for t in range(4):
        maskT[t] = np.where(jl <= il - P * t, f(0.0), f(-1e30))
    iden = np.eye(P, dtype=f)
    return dict(wqT=wqT, wkT=wkT, wvT=wvT, woT=woT, gateBC=gateBC,
                wfcT=wfcT, wprojT=wprojT, maskT=maskT, iden=iden)


_cached_nc = None


def kernel(x, ln1_w, ln2_w, wq, wk, wv, wo, gate_w, w_fc, w_proj):
    global _cached_nc
    x = np.asarray(x, np.float32)
    B = x.shape[0]
    assert x.shape == (B, T, C) and B == 8
    weights = _prep_inputs(x, ln1_w, ln2_w, wq, wk, wv, wo, gate_w, w_fc, w_proj)
    in_maps = [dict(weights, x=np.ascontiguousarray(x[b])) for b in range(B)]
    if _cached_nc is None:
        _cached_nc = build_kernel()
    res = None
    last_err = None
    for attempt in range(3):
        try:
            res = bass_utils.run_bass_kernel_spmd(_cached_nc, in_maps,
                                                  core_ids=list(range(8)))
            break
        except Exception as err:  # transient NRT device errors: retry
            last_err = err
    if res is None:
        raise last_err
    y = np.stack([res.results[b]["y"] for b in range(B)])
    router = np.concatenate([res.results[b]["router"] for b in range(B)], axis=0)
    return y, router


# revision 20
# speedup vs baseline: 1.0349x; 1.0015x over previous
"""Trainium2 Bass kernel for nn_Block_47004122087942 (attention + dense top-2 MoE).

Strategy: pure data-parallel over batch (B=8 -> 8 NeuronCores, zero collectives).
Per core: one batch element [T=1024, C=512].
Compute dtype: float32r (fp32 storage, TF32-like matmul @ full PE rate) everywhere
except the gate-logit path, which runs in exact fp32 to reproduce the reference's
discrete top-2 expert selection.

Layout convention: "T" suffix = transposed [channels-on-partitions, tokens-on-free].
matmul(out[M,N], lhsT[K,M], rhs[K,N]) computes lhsT.T @ rhs with K = partitions.
"""
import numpy as np

import concourse.bass as bass
import concourse.tile as tile
from concourse import bacc, mybir
from concourse import bass_utils

P = 128
T = 1024          # tokens per core
C = 512           # channels
H = 8             # heads
D = 64            # head dim
E = 4             # experts
F = 2048          # ffn dim
NT = T // P       # 8 token tiles
NC = C // P       # 4 channel chunks
NF = F // P       # 16 ffn tiles
TCH = 512         # matmul free-dim chunk of tokens
NTCH = T // TCH   # 2
EPS = 1e-5

f32 = mybir.dt.float32
f32r = mybir.dt.float32r
bf16 = mybir.dt.bfloat16


def build_kernel():
    nc = bacc.Bacc("TRN2", target_bir_lowering=False, debug=False, num_devices=8)

    # ---- DRAM parameters (per-core shard views) ----
    x_ext = nc.dram_tensor("x", [T, C], f32, kind="ExternalInput")
    wqT_ext = nc.dram_tensor("wqT", [NC, P, C], f32, kind="ExternalInput")
    wkT_ext = nc.dram_tensor("wkT", [NC, P, C], f32, kind="ExternalInput")
    wvT_ext = nc.dram_tensor("wvT", [NC, P, C], f32, kind="ExternalInput")
    woT_ext = nc.dram_tensor("woT", [NC, P, C], f32, kind="ExternalInput")
    gateBC_ext = nc.dram_tensor("gateBC", [E, P, C], f32, kind="ExternalInput")
    wfcT_ext = nc.dram_tensor("wfcT", [E, NC, P, F], f32, kind="ExternalInput")
    wprojT_ext = nc.dram_tensor("wprojT", [E, NF, P, C], f32, kind="ExternalInput")
    maskT_ext = nc.dram_tensor("maskT", [4, P, TCH], f32, kind="ExternalInput")
    iden_ext = nc.dram_tensor("iden", [P, P], f32, kind="ExternalInput")
    y_ext = nc.dram_tensor("y", [T, C], f32, kind="ExternalOutput")
    router_ext = nc.dram_tensor("router", [T, E], f32, kind="ExternalOutput")

    rden_dram = [nc.dram_tensor(f"rden_scratch{kk}", [2, T], f32, kind="Internal")
                 for kk in range(NC)]

    with tile.TileContext(nc) as tc:
        from contextlib import ExitStack
        with ExitStack() as ctx:
            const = ctx.enter_context(tc.tile_pool(name="const", bufs=1))
            persist = ctx.enter_context(tc.tile_pool(name="persist", bufs=1))
            small = ctx.enter_context(tc.tile_pool(name="small", bufs=4))
            ntmp = ctx.enter_context(tc.tile_pool(name="ntmp", bufs=2))
            ps_mm = ctx.enter_context(tc.tile_pool(name="ps_mm", bufs=2, space="PSUM"))
            ps_tp = ctx.enter_context(tc.tile_pool(name="ps_tp", bufs=2, space="PSUM"))

            # ---- constants ----
            iden = const.tile([P, P], f32, name="iden")
            nc.sync.dma_start(out=iden[:], in_=iden_ext.ap())
            eps_t = const.tile([P, 1], f32, name="eps")
            nc.vector.memset(eps_t[:], EPS)
            ones8 = const.tile([P, H], f32, name="ones8")
            nc.vector.memset(ones8[:], 1.0)
            iden_r = const.tile([P, P], f32r, name="idenr")
            nc.gpsimd.dma_start(out=iden_r[:], in_=iden_ext.ap())

            # persistent activations
            x_sb = [persist.tile([P, C], f32, name=f"x{m}") for m in range(NT)]
            attT = [persist.tile([P, T], f32r, name=f"attT{k}") for k in range(NC)]
            xn2T_r = [persist.tile([P, T], bf16, name=f"xn2Tr{k}") for k in range(NC)]
            router_sb = [persist.tile([P, E], f32, name=f"rt{m}") for m in range(NT)]

            for m in range(NT):
                nc.sync.dma_start(out=x_sb[m][:], in_=x_ext.ap()[m * P:(m + 1) * P, :])

            # O-proj / gate weights loaded early so stage D can start the moment
            # attT is normalized (no SBUF-address-reuse dependency on the
            # attention scope close)
            woT_sb = []
            for kk in range(NC):
                w = persist.tile([P, C], f32r, name=f"wo{kk}")
                nc.gpsimd.dma_start(out=w[:], in_=woT_ext.ap()[kk])
                woT_sb.append(w)

            def rms_scale(xin, tag):
                """returns [P,1] f32 tile = 1/sqrt(mean(xin^2)+eps)"""
                sq = ntmp.tile([P, C], f32, name="sq")
                nc.vector.tensor_mul(out=sq[:], in0=xin, in1=xin)
                ssq = small.tile([P, 1], f32, name="ssq")
                nc.vector.tensor_reduce(out=ssq[:], in_=sq[:],
                                        axis=mybir.AxisListType.X, op=mybir.AluOpType.add)
                rms = small.tile([P, 1], f32, name="rms")
                nc.scalar.activation(out=rms[:], in_=ssq[:],
                                     func=mybir.ActivationFunctionType.Sqrt,
                                     bias=eps_t[:], scale=1.0 / C)
                rsc = small.tile([P, 1], f32, name="rsc")
                nc.vector.reciprocal(out=rsc[:], in_=rms[:])
                return rsc

            # =========================================================
            # attention scope
            # =========================================================
            with ExitStack() as actx:
                apool = actx.enter_context(tc.tile_pool(name="apool", bufs=1))
                expp = actx.enter_context(tc.tile_pool(name="expp", bufs=3))
                ps_sc = actx.enter_context(tc.tile_pool(name="ps_sc", bufs=2, space="PSUM"))
                ps_pv = actx.enter_context(tc.tile_pool(name="ps_pv", bufs=2, space="PSUM"))

                maskT = []
                for t in range(4):
                    mk = apool.tile([P, TCH], f32r, name=f"mask{t}")
                    nc.gpsimd.dma_start(out=mk[:], in_=maskT_ext.ap()[t])
                    maskT.append(mk)

                # ---- stage A: rms_norm1 + transpose -> xnT (f32r) ----
                bctx = ExitStack()
                bpool = bctx.enter_context(tc.tile_pool(name="bpool", bufs=1))
                xnT = [bpool.tile([P, T], f32r, name=f"xnT{k}") for k in range(NC)]
                for m in range(NT):
                    rsc = rms_scale(x_sb[m][:], f"n1_{m}")
                    xn = ntmp.tile([P, C], f32, name="xn")
                    nc.vector.tensor_scalar_mul(out=xn[:], in0=x_sb[m][:], scalar1=rsc[:])
                    for kk in range(NC):
                        tp = ps_tp.tile([P, P], f32, name="tp")
                        nc.tensor.transpose(tp[:], xn[:, kk * P:(kk + 1) * P], iden[:])
                        nc.scalar.copy(out=xnT[kk][:, m * P:(m + 1) * P], in_=tp[:])

                # ---- stage B: QKV projections ----
                wqT_sb, wkT_sb, wvT_sb = [], [], []
                for kk in range(NC):
                    for lst, ext, nm in ((wqT_sb, wqT_ext, "wq"), (wkT_sb, wkT_ext, "wk"),
                                         (wvT_sb, wvT_ext, "wv")):
                        w = bpool.tile([P, C], f32r, name=f"{nm}{kk}")
                        nc.gpsimd.dma_start(out=w[:], in_=ext.ap()[kk])
                        lst.append(w)

                # q stored per-head, zero-padded to K=128 so the scores matmul
                # can contract over the full 128 partitions (the other head's k
                # rows meet zeros). k stored packed (2 heads per tile).
                zeros512 = apool.tile([P, TCH], f32, name="zeros512")
                nc.vector.memset(zeros512[:], 0.0)
                qz = [apool.tile([P, T], f32r, name=f"qz{h}") for h in range(H)]
                for h in range(H):
                    po = D * (h % 2)
                    zo = D - po  # the half that must be zeroed: 64 if h even else 0
                    for tch in range(NTCH):
                        nc.vector.tensor_copy(
                            out=qz[h][zo:zo + D, tch * TCH:(tch + 1) * TCH],
                            in_=zeros512[0:D, :])
                kT = [apool.tile([P, T], f32r, name=f"kT{o}") for o in range(NC)]
                for o4 in range(NC):
                    for tch in range(NTCH):
                        # K projection (packed)
                        pst = ps_mm.tile([P, TCH], f32, name="mmps")
                        for kk in range(NC):
                            nc.tensor.matmul(
                                pst[:],
                                lhsT=wkT_sb[kk][:, o4 * P:(o4 + 1) * P],
                                rhs=xnT[kk][:, tch * TCH:(tch + 1) * TCH],
                                start=(kk == 0), stop=(kk == NC - 1))
                        nc.scalar.copy(
                            out=kT[o4][:, tch * TCH:(tch + 1) * TCH], in_=pst[:])
                        # Q projection (split per head, placed at matching offset)
                        pst = ps_mm.tile([P, TCH], f32, name="mmps")
                        for kk in range(NC):
                            nc.tensor.matmul(
                                pst[:],
                                lhsT=wqT_sb[kk][:, o4 * P:(o4 + 1) * P],
                                rhs=xnT[kk][:, tch * TCH:(tch + 1) * TCH],
                                start=(kk == 0), stop=(kk == NC - 1))
                        nc.scalar.copy(
                            out=qz[2 * o4][0:D, tch * TCH:(tch + 1) * TCH],
                            in_=pst[0:D, :])
                        nc.scalar.copy(
                            out=qz[2 * o4 + 1][D:P, tch * TCH:(tch + 1) * TCH],
                            in_=pst[D:P, :])

                # V in untransposed layout, interleaved per-head with ones column,
                # zero-padded to a full 128-wide stationary (M=128 matmuls are
                # much faster than M=65 on the fp32r path)
                VW = 128
                zrow = apool.tile([P, VW - D - 1], f32, name="zrow")
                nc.vector.memset(zrow[:], 0.0)
                v_aug = [apool.tile([P, H, VW], f32r, name=f"vaug{m}") for m in range(NT)]
                for m in range(NT):
                    pst = ps_mm.tile([P, C], f32, name="mmps")
                    for kk in range(NC):
                        nc.tensor.matmul(
                            pst[:],
                            lhsT=xnT[kk][:, m * P:(m + 1) * P],
                            rhs=wvT_sb[kk][:],
                            start=(kk == 0), stop=(kk == NC - 1))
                    nc.scalar.copy(
                        out=v_aug[m][:, :, 0:D],
                        in_=pst[:].rearrange("p (h d) -> p h d", h=H))
                    nc.vector.tensor_copy(
                        out=v_aug[m][:, :, D:D + 1].rearrange("p h one -> p (h one)"),
                        in_=ones8[:])
                    for h in range(H):
                        nc.vector.tensor_copy(out=v_aug[m][:, h, D + 1:VW], in_=zrow[:])

                bctx.close()  # free xnT + QKV weights (40KB/partition)
                npool = actx.enter_context(tc.tile_pool(name="npool", bufs=1))

                # ---- stage C: attention, pair-major with pipelined PE emission ----
                for kk2 in range(NC):
                    for hh in range(2):
                        h = 2 * kk2 + hh
                        h2 = h // 2
                        po = D * (h % 2)
                        for ic in range(NTCH):
                            jmax = 4 * (ic + 1)
                            pv = ps_pv.tile([P, TCH], f32, name="pvps")
                            exs = []
                            pending_pv = []
                            for jj in range(jmax):
                                diag = jj >= 4 * ic
                                sc = ps_sc.tile([P, TCH], f32, name="scps")
                                nc.tensor.matmul(
                                    sc[:],
                                    lhsT=kT[h2][:, jj * P:(jj + 1) * P],
                                    rhs=qz[h][:, ic * TCH:(ic + 1) * TCH],
                                    start=True, stop=not diag)
                                if diag:
                                    # accumulate -1e30 additive causal mask on PE
                                    nc.tensor.matmul(
                                        sc[:],
                                        lhsT=iden_r[:],
                                        rhs=maskT[jj - 4 * ic][:],
                                        start=False, stop=True)
                                ex = expp.tile([P, TCH], f32r, name="expT")
                                nc.scalar.activation(out=ex[:], in_=sc[:],
                                                     func=mybir.ActivationFunctionType.Exp)
                                pending_pv.append((jj, ex))
                                # emit pv for the PREVIOUS jj so PE has a score
                                # matmul in flight while ACT computes this exp
                                if len(pending_pv) > 1:
                                    pjj, pex = pending_pv.pop(0)
                                    nc.tensor.matmul(
                                        pv[:, :],
                                        lhsT=v_aug[pjj][:, h, :],
                                        rhs=pex[:],
                                        start=(pjj == 0), stop=False)
                            pjj, pex = pending_pv.pop(0)
                            nc.tensor.matmul(
                                pv[:, :],
                                lhsT=v_aug[pjj][:, h, :],
                                rhs=pex[:],
                                start=(pjj == 0), stop=True)
                            # evict raw attention (f32r) + denominator row
                            nc.vector.tensor_copy(
                                out=attT[h2][po:po + D, ic * TCH:(ic + 1) * TCH],
                                in_=pv[0:D, :])
                            den_t = small.tile([1, TCH], f32, name="den")
                            nc.scalar.copy(out=den_t[:], in_=pv[D:D + 1, :])
                            nc.sync.dma_start(
                                out=rden_dram[kk2].ap()[hh:hh + 1, ic * TCH:(ic + 1) * TCH],
                                in_=den_t[:])
                    # normalize this kk pair immediately (overlaps later heads)
                    bc = npool.tile([P, T], f32, name="bcast")
                    for hh in range(2):
                        srcap = rden_dram[kk2].ap()[hh:hh + 1, :]
                        bsrc = bass.AP(tensor=srcap.tensor, offset=srcap.offset,
                                       ap=[[0, D]] + srcap.ap[1:])
                        nc.sync.dma_start(out=bc[hh * D:(hh + 1) * D, :], in_=bsrc)
                    bc2 = npool.tile([P, T], f32, name="bcrec")
                    scr = npool.tile([P, T], f32, name="bcscr")
                    nc.vector.reciprocal_approx_accurate(out=bc2[:], in_=bc[:], scratch=scr[:])
                    nc.vector.tensor_tensor(out=attT[kk2][:], in0=attT[kk2][:],
                                            in1=bc2[:], op=mybir.AluOpType.mult)

                # =========================================================
                # stage D (inside attention scope: no release barrier between
                # the last PV and the O-projection)
                # =========================================================
                gate_bc = []
                for e in range(E):
                    gb = npool.tile([P, C], f32, name=f"gbc{e}")
                    nc.sync.dma_start(out=gb[:], in_=gateBC_ext.ap()[e])
                    gate_bc.append(gb)

                for m in range(NT):
                    pat = ps_mm.tile([P, C], f32, name="mmps")
                    for kk in range(NC):
                        nc.tensor.matmul(
                            pat[:],
                            lhsT=attT[kk][:, m * P:(m + 1) * P],
                            rhs=woT_sb[kk][:],
                            start=(kk == 0), stop=(kk == NC - 1))
                    # x1 = x + att_out (in place into x_sb)
                    nc.vector.tensor_tensor(out=x_sb[m][:], in0=x_sb[m][:], in1=pat[:],
                                            op=mybir.AluOpType.add)
                    rsc = rms_scale(x_sb[m][:], f"n2_{m}")
                    xn2 = ntmp.tile([P, C], f32, name="xn2")
                    nc.vector.tensor_scalar_mul(out=xn2[:], in0=x_sb[m][:], scalar1=rsc[:])
                    for kk in range(NC):
                        tp = ps_tp.tile([P, P], f32, name="tp")
                        nc.tensor.transpose(tp[:], xn2[:, kk * P:(kk + 1) * P], iden[:])
                        nc.scalar.copy(out=xn2T_r[kk][:, m * P:(m + 1) * P], in_=tp[:])

                    # gate logits in exact fp32 on DVE: fused mult + free-axis
                    # reduce against pre-broadcast gate rows
                    glog = small.tile([P, E], f32, name="glog")
                    for e in range(E):
                        gsc = ntmp.tile([P, C], f32, name="sq")
                        nc.vector.scalar_tensor_tensor(
                            out=gsc[:], in0=xn2[:], scalar=1.0, in1=gate_bc[e][:],
                            op0=mybir.AluOpType.mult, op1=mybir.AluOpType.mult,
                            accum_out=glog[:, e:e + 1])
                    g = glog[:]
                    m1 = small.tile([P, 1], f32, name="m1")
                    nc.vector.tensor_reduce(out=m1[:], in_=g, axis=mybir.AxisListType.X,
                                            op=mybir.AluOpType.max)
                    m1n = small.tile([P, 1], f32, name="m1n")
                    nc.vector.tensor_scalar_mul(out=m1n[:], in0=m1[:], scalar1=-1.0)
                    ex = small.tile([P, E], f32, name="gex")
                    nc.scalar.activation(out=ex[:], in_=g,
                                         func=mybir.ActivationFunctionType.Exp,
                                         bias=m1n[:], scale=1.0)
                    eq = small.tile([P, E], f32, name="geq")
                    nc.vector.tensor_scalar(out=eq[:], in0=g, scalar1=m1[:], scalar2=None,
                                            op0=mybir.AluOpType.is_ge)
                    gm = small.tile([P, E], f32, name="ggm")
                    nc.vector.scalar_tensor_tensor(out=gm[:], in0=eq[:], scalar=-1e30,
                                                   in1=g, op0=mybir.AluOpType.mult,
                                                   op1=mybir.AluOpType.add)
                    m2 = small.tile([P, 1], f32, name="m2")
                    nc.vector.tensor_reduce(out=m2[:], in_=gm[:], axis=mybir.AxisListType.X,
                                            op=mybir.AluOpType.max)
                    keep = small.tile([P, E], f32, name="gkeep")
                    nc.vector.tensor_scalar(out=keep[:], in0=g, scalar1=m2[:], scalar2=None,
                                            op0=mybir.AluOpType.is_ge)
                    wun = small.tile([P, E], f32, name="gwun")
                    nc.vector.tensor_tensor(out=wun[:], in0=ex[:], in1=keep[:],
                                            op=mybir.AluOpType.mult)
                    s = small.tile([P, 1], f32, name="gsum")
                    nc.vector.tensor_reduce(out=s[:], in_=wun[:], axis=mybir.AxisListType.X,
                                            op=mybir.AluOpType.add)
                    rs = small.tile([P, 1], f32, name="grs")
                    nc.vector.reciprocal(out=rs[:], in_=s[:])
                    nc.vector.tensor_scalar_mul(out=router_sb[m][:], in0=wun[:], scalar1=rs[:])
                    nc.sync.dma_start(out=router_ext.ap()[m * P:(m + 1) * P, :],
                                      in_=router_sb[m][:])
            # attention + stage D scope closed

            # =========================================================
            # stage E scope: MoE pools
            # =========================================================
            with ExitStack() as dctx:
                wfcp = dctx.enter_context(tc.tile_pool(name="wfcp", bufs=2))
                wprp = dctx.enter_context(tc.tile_pool(name="wprp", bufs=1))
                hp = dctx.enter_context(tc.tile_pool(name="hp", bufs=3))
                ps_o = dctx.enter_context(tc.tile_pool(name="ps_o", bufs=4, space="PSUM"))

                def load_expert(e):
                    fc, pr = [], []
                    for kk in range(NC):
                        w = wfcp.tile([P, F], bf16, name=f"wfc{kk}")
                        nc.gpsimd.dma_start(out=w[:], in_=wfcT_ext.ap()[e, kk])
                        fc.append(w)
                    for k in range(NF):
                        w = wprp.tile([P, C], bf16, name=f"wpr{k}")
                        nc.gpsimd.dma_start(out=w[:], in_=wprojT_ext.ap()[e, k])
                        pr.append(w)
                    return fc, pr

                next_w = load_expert(0)

                # =========================================================
                for e in range(E):
                    wfc_sb, wpr_sb = next_w
                    if e + 1 < E:
                        next_w = load_expert(e + 1)
                    for tch in range(NTCH):
                        po_tiles = [ps_o.tile([P, C], f32, name="moeo")
                                    for i in range(4)]
                        pending = []
                        for k in range(NF):
                            ph = ps_mm.tile([P, TCH], f32, name="mmps")
                            for kk in range(NC):
                                nc.tensor.matmul(
                                    ph[:],
                                    lhsT=wfc_sb[kk][:, k * P:(k + 1) * P],
                                    rhs=xn2T_r[kk][:, tch * TCH:(tch + 1) * TCH],
                                    start=(kk == 0), stop=(kk == NC - 1))
                            hT = hp.tile([P, TCH], bf16, name="hT")
                            nc.scalar.activation(out=hT[:], in_=ph[:],
                                                 func=mybir.ActivationFunctionType.Gelu)
                            pending.append((k, hT))
                            if len(pending) > 1:
                                pk, phT = pending.pop(0)
                                for i in range(4):
                                    nc.tensor.matmul(
                                        po_tiles[i][:],
                                        lhsT=phT[:, i * P:(i + 1) * P],
                                        rhs=wpr_sb[pk][:],
                                        start=(pk == 0), stop=False)
                        pk, phT = pending.pop(0)
                        for i in range(4):
                            nc.tensor.matmul(
                                po_tiles[i][:],
                                lhsT=phT[:, i * P:(i + 1) * P],
                                rhs=wpr_sb[pk][:],
                                start=(pk == 0), stop=True)
                        for i in range(4):
                            m = 4 * tch + i
                            nc.vector.scalar_tensor_tensor(
                                out=x_sb[m][:], in0=po_tiles[i][:],
                                scalar=router_sb[m][:, e:e + 1], in1=x_sb[m][:],
                                op0=mybir.AluOpType.mult, op1=mybir.AluOpType.add)

            for m in range(NT):
                nc.sync.dma_start(out=y_ext.ap()[m * P:(m + 1) * P, :], in_=x_sb[m][:])

    nc.compile()
    return nc


def _prep_inputs(x, ln1_w, ln2_w, wq, wk, wv, wo, gate_w, w_fc, w_proj):
    """Host-side weight preprocessing: fold rms-norm gains + score scale,
    pre-transpose for TensorE (contraction dim on partitions)."""
    f = np.float32
    ln1 = np.asarray(ln1_w, f)
    ln2 = np.asarray(ln2_w, f)
    wqT = np.ascontiguousarray((np.asarray(wq, f) * ln1[None, :]).T).reshape(NC, P, C)
    wkT = np.ascontiguousarray((np.asarray(wk, f) * ln1[None, :]).T / np.sqrt(np.float32(D))).reshape(NC, P, C)
    wvT = np.ascontiguousarray((np.asarray(wv, f) * ln1[None, :]).T).reshape(NC, P, C)
    woT = np.ascontiguousarray(np.asarray(wo, f).T).reshape(NC, P, C)
    gateBC = np.ascontiguousarray(
        np.repeat((np.asarray(gate_w, f) * ln2[None, :])[:, None, :], P, axis=1))
    wfcT = np.stack([
        np.ascontiguousarray((np.asarray(w_fc, f)[e] * ln2[None, :]).T).reshape(NC, P, F)
        for e in range(E)])
    wprojT = np.stack([
        np.ascontiguousarray(np.asarray(w_proj, f)[e].T).reshape(NF, P, C)
        for e in range(E)])
    # additive causal masks in transposed-scores layout: tile covers j in
    # [128t, 128t+128), i in [0, 512) relative; keep (0.0) iff
    # j_local <= i_local - 128t else -1e30 (exp -> 0)
    maskT = np.zeros((4, P, TCH), f)
    jl = np.arange(P)[:, None]
    il = np.arange(TCH)[None, :]
    for t in range(4):
        maskT[t] = np.where(jl <= il - P * t, f(0.0), f(-1e30))
    iden = np.eye(P, dtype=f)
    return dict(wqT=wqT, wkT=wkT, wvT=wvT, woT=woT, gateBC=gateBC,
                wfcT=wfcT, wprojT=wprojT, maskT=maskT, iden=iden)


_cached_nc = None


def kernel(x, ln1_w, ln2_w, wq, wk, wv, wo, gate_w, w_fc, w_proj):
    global _cached_nc
    x = np.asarray(x, np.float32)
    B = x.shape[0]
    assert x.shape == (B, T, C) and B == 8
    weights = _prep_inputs(x, ln1_w, ln2_w, wq, wk, wv, wo, gate_w, w_fc, w_proj)
    in_maps = [dict(weights, x=np.ascontiguousarray(x[b])) for b in range(B)]
    if _cached_nc is None:
        _cached_nc = build_kernel()
    res = None
    last_err = None
    for attempt in range(3):
        try:
            res = bass_utils.run_bass_kernel_spmd(_cached_nc, in_maps,
                                                  core_ids=list(range(8)))
            break
        except Exception as err:  # transient NRT device errors: retry
            last_err = err
    if res is None:
        raise last_err
    y = np.stack([res.results[b]["y"] for b in range(B)])
    router = np.concatenate([res.results[b]["router"] for b in range(B)], axis=0)
    return y, router


# revision 21
# speedup vs baseline: 1.0492x; 1.0138x over previous
"""Trainium2 Bass kernel for nn_Block_47004122087942 (attention + dense top-2 MoE).

Strategy: pure data-parallel over batch (B=8 -> 8 NeuronCores, zero collectives).
Per core: one batch element [T=1024, C=512].
Compute dtype: float32r (fp32 storage, TF32-like matmul @ full PE rate) everywhere
except the gate-logit path, which runs in exact fp32 to reproduce the reference's
discrete top-2 expert selection.

Layout convention: "T" suffix = transposed [channels-on-partitions, tokens-on-free].
matmul(out[M,N], lhsT[K,M], rhs[K,N]) computes lhsT.T @ rhs with K = partitions.
"""
import numpy as np
import ml_dtypes

import concourse.bass as bass
import concourse.tile as tile
from concourse import bacc, mybir
from concourse import bass_utils

P = 128
T = 1024          # tokens per core
C = 512           # channels
H = 8             # heads
D = 64            # head dim
E = 4             # experts
F = 2048          # ffn dim
NT = T // P       # 8 token tiles
NC = C // P       # 4 channel chunks
NF = F // P       # 16 ffn tiles
TCH = 512         # matmul free-dim chunk of tokens
NTCH = T // TCH   # 2
EPS = 1e-5

f32 = mybir.dt.float32
f32r = mybir.dt.float32r
bf16 = mybir.dt.bfloat16


def build_kernel():
    nc = bacc.Bacc("TRN2", target_bir_lowering=False, debug=False, num_devices=8)

    # ---- DRAM parameters (per-core shard views) ----
    x_ext = nc.dram_tensor("x", [T, C], f32, kind="ExternalInput")
    wqT_ext = nc.dram_tensor("wqT", [NC, P, C], f32, kind="ExternalInput")
    wkT_ext = nc.dram_tensor("wkT", [NC, P, C], f32, kind="ExternalInput")
    wvT_ext = nc.dram_tensor("wvT", [NC, P, C], f32, kind="ExternalInput")
    woT_ext = nc.dram_tensor("woT", [NC, P, C], f32, kind="ExternalInput")
    gateBC_ext = nc.dram_tensor("gateBC", [E, P, C], f32, kind="ExternalInput")
    wfcT_ext = nc.dram_tensor("wfcT", [E, NC, P, F], bf16, kind="ExternalInput")
    wprojT_ext = nc.dram_tensor("wprojT", [E, NF, P, C], bf16, kind="ExternalInput")
    maskT_ext = nc.dram_tensor("maskT", [4, P, TCH], f32, kind="ExternalInput")
    iden_ext = nc.dram_tensor("iden", [P, P], f32, kind="ExternalInput")
    y_ext = nc.dram_tensor("y", [T, C], f32, kind="ExternalOutput")
    router_ext = nc.dram_tensor("router", [T, E], f32, kind="ExternalOutput")

    rden_dram = [nc.dram_tensor(f"rden_scratch{kk}", [2, T], f32, kind="Internal")
                 for kk in range(NC)]

    with tile.TileContext(nc) as tc:
        from contextlib import ExitStack
        with ExitStack() as ctx:
            const = ctx.enter_context(tc.tile_pool(name="const", bufs=1))
            persist = ctx.enter_context(tc.tile_pool(name="persist", bufs=1))
            small = ctx.enter_context(tc.tile_pool(name="small", bufs=4))
            ntmp = ctx.enter_context(tc.tile_pool(name="ntmp", bufs=2))
            ps_mm = ctx.enter_context(tc.tile_pool(name="ps_mm", bufs=2, space="PSUM"))
            ps_tp = ctx.enter_context(tc.tile_pool(name="ps_tp", bufs=2, space="PSUM"))

            # ---- constants ----
            iden = const.tile([P, P], f32, name="iden")
            nc.sync.dma_start(out=iden[:], in_=iden_ext.ap())
            eps_t = const.tile([P, 1], f32, name="eps")
            nc.vector.memset(eps_t[:], EPS)
            ones8 = const.tile([P, H], f32, name="ones8")
            nc.vector.memset(ones8[:], 1.0)
            iden_r = const.tile([P, P], f32r, name="idenr")
            nc.gpsimd.dma_start(out=iden_r[:], in_=iden_ext.ap())

            # persistent activations
            x_sb = [persist.tile([P, C], f32, name=f"x{m}") for m in range(NT)]
            attT = [persist.tile([P, T], f32r, name=f"attT{k}") for k in range(NC)]
            xn2T_r = [persist.tile([P, T], bf16, name=f"xn2Tr{k}") for k in range(NC)]
            router_sb = [persist.tile([P, E], f32, name=f"rt{m}") for m in range(NT)]

            for m in range(NT):
                nc.sync.dma_start(out=x_sb[m][:], in_=x_ext.ap()[m * P:(m + 1) * P, :])

            # O-proj / gate weights loaded early so stage D can start the moment
            # attT is normalized (no SBUF-address-reuse dependency on the
            # attention scope close)
            woT_sb = []
            for kk in range(NC):
                w = persist.tile([P, C], f32r, name=f"wo{kk}")
                nc.gpsimd.dma_start(out=w[:], in_=woT_ext.ap()[kk])
                woT_sb.append(w)

            def rms_scale(xin, tag):
                """returns [P,1] f32 tile = 1/sqrt(mean(xin^2)+eps)"""
                sq = ntmp.tile([P, C], f32, name="sq")
                nc.vector.tensor_mul(out=sq[:], in0=xin, in1=xin)
                ssq = small.tile([P, 1], f32, name="ssq")
                nc.vector.tensor_reduce(out=ssq[:], in_=sq[:],
                                        axis=mybir.AxisListType.X, op=mybir.AluOpType.add)
                rms = small.tile([P, 1], f32, name="rms")
                nc.scalar.activation(out=rms[:], in_=ssq[:],
                                     func=mybir.ActivationFunctionType.Sqrt,
                                     bias=eps_t[:], scale=1.0 / C)
                rsc = small.tile([P, 1], f32, name="rsc")
                nc.vector.reciprocal(out=rsc[:], in_=rms[:])
                return rsc

            # =========================================================
            # attention scope
            # =========================================================
            with ExitStack() as actx:
                apool = actx.enter_context(tc.tile_pool(name="apool", bufs=1))
                expp = actx.enter_context(tc.tile_pool(name="expp", bufs=3))
                ps_sc = actx.enter_context(tc.tile_pool(name="ps_sc", bufs=2, space="PSUM"))
                ps_pv = actx.enter_context(tc.tile_pool(name="ps_pv", bufs=2, space="PSUM"))

                maskT = []
                for t in range(4):
                    mk = apool.tile([P, TCH], f32r, name=f"mask{t}")
                    nc.gpsimd.dma_start(out=mk[:], in_=maskT_ext.ap()[t])
                    maskT.append(mk)

                # ---- stage A: rms_norm1 + transpose -> xnT (f32r) ----
                bctx = ExitStack()
                bpool = bctx.enter_context(tc.tile_pool(name="bpool", bufs=1))
                xnT = [bpool.tile([P, T], f32r, name=f"xnT{k}") for k in range(NC)]
                for m in range(NT):
                    rsc = rms_scale(x_sb[m][:], f"n1_{m}")
                    xn = ntmp.tile([P, C], f32, name="xn")
                    nc.vector.tensor_scalar_mul(out=xn[:], in0=x_sb[m][:], scalar1=rsc[:])
                    for kk in range(NC):
                        tp = ps_tp.tile([P, P], f32, name="tp")
                        nc.tensor.transpose(tp[:], xn[:, kk * P:(kk + 1) * P], iden[:])
                        nc.scalar.copy(out=xnT[kk][:, m * P:(m + 1) * P], in_=tp[:])

                # ---- stage B: QKV projections ----
                wqT_sb, wkT_sb, wvT_sb = [], [], []
                for kk in range(NC):
                    for lst, ext, nm in ((wqT_sb, wqT_ext, "wq"), (wkT_sb, wkT_ext, "wk"),
                                         (wvT_sb, wvT_ext, "wv")):
                        w = bpool.tile([P, C], f32r, name=f"{nm}{kk}")
                        nc.gpsimd.dma_start(out=w[:], in_=ext.ap()[kk])
                        lst.append(w)

                # q stored per-head, zero-padded to K=128 so the scores matmul
                # can contract over the full 128 partitions (the other head's k
                # rows meet zeros). k stored packed (2 heads per tile).
                zeros512 = apool.tile([P, TCH], f32, name="zeros512")
                nc.vector.memset(zeros512[:], 0.0)
                qz = [apool.tile([P, T], f32r, name=f"qz{h}") for h in range(H)]
                for h in range(H):
                    po = D * (h % 2)
                    zo = D - po  # the half that must be zeroed: 64 if h even else 0
                    for tch in range(NTCH):
                        nc.vector.tensor_copy(
                            out=qz[h][zo:zo + D, tch * TCH:(tch + 1) * TCH],
                            in_=zeros512[0:D, :])
                kT = [apool.tile([P, T], f32r, name=f"kT{o}") for o in range(NC)]
                for o4 in range(NC):
                    for tch in range(NTCH):
                        # K projection (packed)
                        pst = ps_mm.tile([P, TCH], f32, name="mmps")
                        for kk in range(NC):
                            nc.tensor.matmul(
                                pst[:],
                                lhsT=wkT_sb[kk][:, o4 * P:(o4 + 1) * P],
                                rhs=xnT[kk][:, tch * TCH:(tch + 1) * TCH],
                                start=(kk == 0), stop=(kk == NC - 1))
                        nc.scalar.copy(
                            out=kT[o4][:, tch * TCH:(tch + 1) * TCH], in_=pst[:])
                        # Q projection (split per head, placed at matching offset)
                        pst = ps_mm.tile([P, TCH], f32, name="mmps")
                        for kk in range(NC):
                            nc.tensor.matmul(
                                pst[:],
                                lhsT=wqT_sb[kk][:, o4 * P:(o4 + 1) * P],
                                rhs=xnT[kk][:, tch * TCH:(tch + 1) * TCH],
                                start=(kk == 0), stop=(kk == NC - 1))
                        nc.scalar.copy(
                            out=qz[2 * o4][0:D, tch * TCH:(tch + 1) * TCH],
                            in_=pst[0:D, :])
                        nc.scalar.copy(
                            out=qz[2 * o4 + 1][D:P, tch * TCH:(tch + 1) * TCH],
                            in_=pst[D:P, :])

                # V in untransposed layout, interleaved per-head with ones column,
                # zero-padded to a full 128-wide stationary (M=128 matmuls are
                # much faster than M=65 on the fp32r path)
                VW = 128
                zrow = apool.tile([P, VW - D - 1], f32, name="zrow")
                nc.vector.memset(zrow[:], 0.0)
                v_aug = [apool.tile([P, H, VW], f32r, name=f"vaug{m}") for m in range(NT)]
                for m in range(NT):
                    pst = ps_mm.tile([P, C], f32, name="mmps")
                    for kk in range(NC):
                        nc.tensor.matmul(
                            pst[:],
                            lhsT=xnT[kk][:, m * P:(m + 1) * P],
                            rhs=wvT_sb[kk][:],
                            start=(kk == 0), stop=(kk == NC - 1))
                    nc.scalar.copy(
                        out=v_aug[m][:, :, 0:D],
                        in_=pst[:].rearrange("p (h d) -> p h d", h=H))
                    nc.vector.tensor_copy(
                        out=v_aug[m][:, :, D:D + 1].rearrange("p h one -> p (h one)"),
                        in_=ones8[:])
                    for h in range(H):
                        nc.vector.tensor_copy(out=v_aug[m][:, h, D + 1:VW], in_=zrow[:])

                bctx.close()  # free xnT + QKV weights (40KB/partition)
                npool = actx.enter_context(tc.tile_pool(name="npool", bufs=1))

                # ---- stage C: attention, pair-major with pipelined PE emission ----
                for kk2 in range(NC):
                    for hh in range(2):
                        h = 2 * kk2 + hh
                        h2 = h // 2
                        po = D * (h % 2)
                        for ic in range(NTCH):
                            jmax = 4 * (ic + 1)
                            pv = ps_pv.tile([P, TCH], f32, name="pvps")
                            exs = []
                            pending_pv = []
                            for jj in range(jmax):
                                diag = jj >= 4 * ic
                                sc = ps_sc.tile([P, TCH], f32, name="scps")
                                nc.tensor.matmul(
                                    sc[:],
                                    lhsT=kT[h2][:, jj * P:(jj + 1) * P],
                                    rhs=qz[h][:, ic * TCH:(ic + 1) * TCH],
                                    start=True, stop=not diag)
                                if diag:
                                    # accumulate -1e30 additive causal mask on PE
                                    nc.tensor.matmul(
                                        sc[:],
                                        lhsT=iden_r[:],
                                        rhs=maskT[jj - 4 * ic][:],
                                        start=False, stop=True)
                                ex = expp.tile([P, TCH], f32r, name="expT")
                                nc.scalar.activation(out=ex[:], in_=sc[:],
                                                     func=mybir.ActivationFunctionType.Exp)
                                pending_pv.append((jj, ex))
                                # emit pv for the PREVIOUS jj so PE has a score
                                # matmul in flight while ACT computes this exp
                                if len(pending_pv) > 1:
                                    pjj, pex = pending_pv.pop(0)
                                    nc.tensor.matmul(
                                        pv[:, :],
                                        lhsT=v_aug[pjj][:, h, :],
                                        rhs=pex[:],
                                        start=(pjj == 0), stop=False)
                            pjj, pex = pending_pv.pop(0)
                            nc.tensor.matmul(
                                pv[:, :],
                                lhsT=v_aug[pjj][:, h, :],
                                rhs=pex[:],
                                start=(pjj == 0), stop=True)
                            # evict raw attention (f32r) + denominator row
                            nc.vector.tensor_copy(
                                out=attT[h2][po:po + D, ic * TCH:(ic + 1) * TCH],
                                in_=pv[0:D, :])
                            den_t = small.tile([1, TCH], f32, name="den")
                            nc.scalar.copy(out=den_t[:], in_=pv[D:D + 1, :])
                            nc.sync.dma_start(
                                out=rden_dram[kk2].ap()[hh:hh + 1, ic * TCH:(ic + 1) * TCH],
                                in_=den_t[:])
                    # normalize this kk pair immediately (overlaps later heads)
                    bc = npool.tile([P, T], f32, name="bcast")
                    for hh in range(2):
                        srcap = rden_dram[kk2].ap()[hh:hh + 1, :]
                        bsrc = bass.AP(tensor=srcap.tensor, offset=srcap.offset,
                                       ap=[[0, D]] + srcap.ap[1:])
                        nc.sync.dma_start(out=bc[hh * D:(hh + 1) * D, :], in_=bsrc)
                    bc2 = npool.tile([P, T], f32, name="bcrec")
                    scr = npool.tile([P, T], f32, name="bcscr")
                    nc.vector.reciprocal_approx_accurate(out=bc2[:], in_=bc[:], scratch=scr[:])
                    nc.vector.tensor_tensor(out=attT[kk2][:], in0=attT[kk2][:],
                                            in1=bc2[:], op=mybir.AluOpType.mult)

                # =========================================================
                # stage D (inside attention scope: no release barrier between
                # the last PV and the O-projection)
                # =========================================================
                gate_bc = []
                for e in range(E):
                    gb = npool.tile([P, C], f32, name=f"gbc{e}")
                    nc.sync.dma_start(out=gb[:], in_=gateBC_ext.ap()[e])
                    gate_bc.append(gb)

                for m in range(NT):
                    pat = ps_mm.tile([P, C], f32, name="mmps")
                    for kk in range(NC):
                        nc.tensor.matmul(
                            pat[:],
                            lhsT=attT[kk][:, m * P:(m + 1) * P],
                            rhs=woT_sb[kk][:],
                            start=(kk == 0), stop=(kk == NC - 1))
                    # x1 = x + att_out (in place into x_sb)
                    nc.vector.tensor_tensor(out=x_sb[m][:], in0=x_sb[m][:], in1=pat[:],
                                            op=mybir.AluOpType.add)
                    rsc = rms_scale(x_sb[m][:], f"n2_{m}")
                    xn2 = ntmp.tile([P, C], f32, name="xn2")
                    nc.vector.tensor_scalar_mul(out=xn2[:], in0=x_sb[m][:], scalar1=rsc[:])
                    for kk in range(NC):
                        tp = ps_tp.tile([P, P], f32, name="tp")
                        nc.tensor.transpose(tp[:], xn2[:, kk * P:(kk + 1) * P], iden[:])
                        nc.scalar.copy(out=xn2T_r[kk][:, m * P:(m + 1) * P], in_=tp[:])

                    # gate logits in exact fp32 on DVE: fused mult + free-axis
                    # reduce against pre-broadcast gate rows
                    glog = small.tile([P, E], f32, name="glog")
                    for e in range(E):
                        gsc = ntmp.tile([P, C], f32, name="sq")
                        nc.vector.scalar_tensor_tensor(
                            out=gsc[:], in0=xn2[:], scalar=1.0, in1=gate_bc[e][:],
                            op0=mybir.AluOpType.mult, op1=mybir.AluOpType.mult,
                            accum_out=glog[:, e:e + 1])
                    g = glog[:]
                    m1 = small.tile([P, 1], f32, name="m1")
                    nc.vector.tensor_reduce(out=m1[:], in_=g, axis=mybir.AxisListType.X,
                                            op=mybir.AluOpType.max)
                    m1n = small.tile([P, 1], f32, name="m1n")
                    nc.vector.tensor_scalar_mul(out=m1n[:], in0=m1[:], scalar1=-1.0)
                    ex = small.tile([P, E], f32, name="gex")
                    nc.scalar.activation(out=ex[:], in_=g,
                                         func=mybir.ActivationFunctionType.Exp,
                                         bias=m1n[:], scale=1.0)
                    eq = small.tile([P, E], f32, name="geq")
                    nc.vector.tensor_scalar(out=eq[:], in0=g, scalar1=m1[:], scalar2=None,
                                            op0=mybir.AluOpType.is_ge)
                    gm = small.tile([P, E], f32, name="ggm")
                    nc.vector.scalar_tensor_tensor(out=gm[:], in0=eq[:], scalar=-1e30,
                                                   in1=g, op0=mybir.AluOpType.mult,
                                                   op1=mybir.AluOpType.add)
                    m2 = small.tile([P, 1], f32, name="m2")
                    nc.vector.tensor_reduce(out=m2[:], in_=gm[:], axis=mybir.AxisListType.X,
                                            op=mybir.AluOpType.max)
                    keep = small.tile([P, E], f32, name="gkeep")
                    nc.vector.tensor_scalar(out=keep[:], in0=g, scalar1=m2[:], scalar2=None,
                                            op0=mybir.AluOpType.is_ge)
                    wun = small.tile([P, E], f32, name="gwun")
                    nc.vector.tensor_tensor(out=wun[:], in0=ex[:], in1=keep[:],
                                            op=mybir.AluOpType.mult)
                    s = small.tile([P, 1], f32, name="gsum")
                    nc.vector.tensor_reduce(out=s[:], in_=wun[:], axis=mybir.AxisListType.X,
                                            op=mybir.AluOpType.add)
                    rs = small.tile([P, 1], f32, name="grs")
                    nc.vector.reciprocal(out=rs[:], in_=s[:])
                    nc.vector.tensor_scalar_mul(out=router_sb[m][:], in0=wun[:], scalar1=rs[:])
                    nc.sync.dma_start(out=router_ext.ap()[m * P:(m + 1) * P, :],
                                      in_=router_sb[m][:])
            # attention + stage D scope closed

            # =========================================================
            # stage E scope: MoE pools
            # =========================================================
            with ExitStack() as dctx:
                wfcp = dctx.enter_context(tc.tile_pool(name="wfcp", bufs=2))
                wprp = dctx.enter_context(tc.tile_pool(name="wprp", bufs=1))
                hp = dctx.enter_context(tc.tile_pool(name="hp", bufs=3))
                ps_o = dctx.enter_context(tc.tile_pool(name="ps_o", bufs=4, space="PSUM"))

                def load_expert(e):
                    fc, pr = [], []
                    for kk in range(NC):
                        w = wfcp.tile([P, F], bf16, name=f"wfc{kk}")
                        nc.sync.dma_start(out=w[:], in_=wfcT_ext.ap()[e, kk])
                        fc.append(w)
                    for k in range(NF):
                        w = wprp.tile([P, C], bf16, name=f"wpr{k}")
                        nc.sync.dma_start(out=w[:], in_=wprojT_ext.ap()[e, k])
                        pr.append(w)
                    return fc, pr

                next_w = load_expert(0)

                # =========================================================
                for e in range(E):
                    wfc_sb, wpr_sb = next_w
                    if e + 1 < E:
                        next_w = load_expert(e + 1)
                    for tch in range(NTCH):
                        po_tiles = [ps_o.tile([P, C], f32, name="moeo")
                                    for i in range(4)]
                        pending = []
                        for k in range(NF):
                            ph = ps_mm.tile([P, TCH], f32, name="mmps")
                            for kk in range(NC):
                                nc.tensor.matmul(
                                    ph[:],
                                    lhsT=wfc_sb[kk][:, k * P:(k + 1) * P],
                                    rhs=xn2T_r[kk][:, tch * TCH:(tch + 1) * TCH],
                                    start=(kk == 0), stop=(kk == NC - 1))
                            hT = hp.tile([P, TCH], bf16, name="hT")
                            nc.scalar.activation(out=hT[:], in_=ph[:],
                                                 func=mybir.ActivationFunctionType.Gelu)
                            pending.append((k, hT))
                            if len(pending) > 1:
                                pk, phT = pending.pop(0)
                                for i in range(4):
                                    nc.tensor.matmul(
                                        po_tiles[i][:],
                                        lhsT=phT[:, i * P:(i + 1) * P],
                                        rhs=wpr_sb[pk][:],
                                        start=(pk == 0), stop=False)
                        pk, phT = pending.pop(0)
                        for i in range(4):
                            nc.tensor.matmul(
                                po_tiles[i][:],
                                lhsT=phT[:, i * P:(i + 1) * P],
                                rhs=wpr_sb[pk][:],
                                start=(pk == 0), stop=True)
                        for i in range(4):
                            m = 4 * tch + i
                            nc.vector.scalar_tensor_tensor(
                                out=x_sb[m][:], in0=po_tiles[i][:],
                                scalar=router_sb[m][:, e:e + 1], in1=x_sb[m][:],
                                op0=mybir.AluOpType.mult, op1=mybir.AluOpType.add)

            for m in range(NT):
                nc.sync.dma_start(out=y_ext.ap()[m * P:(m + 1) * P, :], in_=x_sb[m][:])

    nc.compile()
    return nc


def _prep_inputs(x, ln1_w, ln2_w, wq, wk, wv, wo, gate_w, w_fc, w_proj):
    """Host-side weight preprocessing: fold rms-norm gains + score scale,
    pre-transpose for TensorE (contraction dim on partitions)."""
    f = np.float32
    ln1 = np.asarray(ln1_w, f)
    ln2 = np.asarray(ln2_w, f)
    wqT = np.ascontiguousarray((np.asarray(wq, f) * ln1[None, :]).T).reshape(NC, P, C)
    wkT = np.ascontiguousarray((np.asarray(wk, f) * ln1[None, :]).T / np.sqrt(np.float32(D))).reshape(NC, P, C)
    wvT = np.ascontiguousarray((np.asarray(wv, f) * ln1[None, :]).T).reshape(NC, P, C)
    woT = np.ascontiguousarray(np.asarray(wo, f).T).reshape(NC, P, C)
    gateBC = np.ascontiguousarray(
        np.repeat((np.asarray(gate_w, f) * ln2[None, :])[:, None, :], P, axis=1))
    wfcT = np.stack([
        np.ascontiguousarray((np.asarray(w_fc, f)[e] * ln2[None, :]).T).reshape(NC, P, F)
        for e in range(E)]).astype(ml_dtypes.bfloat16)
    wprojT = np.stack([
        np.ascontiguousarray(np.asarray(w_proj, f)[e].T).reshape(NF, P, C)
        for e in range(E)]).astype(ml_dtypes.bfloat16)
    # additive causal masks in transposed-scores layout: tile covers j in
    # [128t, 128t+128), i in [0, 512) relative; keep (0.0) iff
    # j_local <= i_local - 128t else -1e30 (exp -> 0)
    maskT = np.zeros((4, P, TCH), f)
    jl = np.arange(P)[:, None]
    il = np.arange(TCH)[None, :]
    for t in range(4):
        maskT[t] = np.where(jl <= il - P * t, f(0.0), f(-1e30))
    iden = np.eye(P, dtype=f)
    return dict(wqT=wqT, wkT=wkT, wvT=wvT, woT=woT, gateBC=gateBC,
                wfcT=wfcT, wprojT=wprojT, maskT=maskT, iden=iden)


_cached_nc = None


def kernel(x, ln1_w, ln2_w, wq, wk, wv, wo, gate_w, w_fc, w_proj):
    global _cached_nc
    x = np.asarray(x, np.float32)
    B = x.shape[0]
    assert x.shape == (B, T, C) and B == 8
    weights = _prep_inputs(x, ln1_w, ln2_w, wq, wk, wv, wo, gate_w, w_fc, w_proj)
    in_maps = [dict(weights, x=np.ascontiguousarray(x[b])) for b in range(B)]
    if _cached_nc is None:
        _cached_nc = build_kernel()
    res = None
    last_err = None
    for attempt in range(3):
        try:
            res = bass_utils.run_bass_kernel_spmd(_cached_nc, in_maps,
                                                  core_ids=list(range(8)))
            break
        except Exception as err:  # transient NRT device errors: retry
            last_err = err
    if res is None:
        raise last_err
    y = np.stack([res.results[b]["y"] for b in range(B)])
    router = np.concatenate([res.results[b]["router"] for b in range(B)], axis=0)
    return y, router


# revision 22
# speedup vs baseline: 1.0851x; 1.0342x over previous
"""Trainium2 Bass kernel for nn_Block_47004122087942 (attention + dense top-2 MoE).

Strategy: pure data-parallel over batch (B=8 -> 8 NeuronCores, zero collectives).
Per core: one batch element [T=1024, C=512].
Compute dtype: float32r (fp32 storage, TF32-like matmul @ full PE rate) everywhere
except the gate-logit path, which runs in exact fp32 to reproduce the reference's
discrete top-2 expert selection.

Layout convention: "T" suffix = transposed [channels-on-partitions, tokens-on-free].
matmul(out[M,N], lhsT[K,M], rhs[K,N]) computes lhsT.T @ rhs with K = partitions.
"""
import numpy as np
import ml_dtypes

import concourse.bass as bass
import concourse.tile as tile
from concourse import bacc, mybir
from concourse import bass_utils

P = 128
T = 1024          # tokens per core
C = 512           # channels
H = 8             # heads
D = 64            # head dim
E = 4             # experts
F = 2048          # ffn dim
NT = T // P       # 8 token tiles
NC = C // P       # 4 channel chunks
NF = F // P       # 16 ffn tiles
TCH = 512         # matmul free-dim chunk of tokens
NTCH = T // TCH   # 2
EPS = 1e-5

f32 = mybir.dt.float32
f32r = mybir.dt.float32r
bf16 = mybir.dt.bfloat16


def build_kernel():
    nc = bacc.Bacc("TRN2", target_bir_lowering=False, debug=False, num_devices=8)

    # ---- DRAM parameters (per-core shard views) ----
    x_ext = nc.dram_tensor("x", [T, C], f32, kind="ExternalInput")
    wqT_ext = nc.dram_tensor("wqT", [NC, P, C], f32, kind="ExternalInput")
    wkT_ext = nc.dram_tensor("wkT", [NC, P, C], f32, kind="ExternalInput")
    wvT_ext = nc.dram_tensor("wvT", [NC, P, C], f32, kind="ExternalInput")
    woT_ext = nc.dram_tensor("woT", [NC, P, C], f32, kind="ExternalInput")
    gateBC_ext = nc.dram_tensor("gateBC", [E, P, C], f32, kind="ExternalInput")
    wfcT_ext = nc.dram_tensor("wfcT", [E, NC, P, F], bf16, kind="ExternalInput")
    wprojT_ext = nc.dram_tensor("wprojT", [E, NF, P, C], bf16, kind="ExternalInput")
    maskT_ext = nc.dram_tensor("maskT", [4, P, TCH], f32, kind="ExternalInput")
    iden_ext = nc.dram_tensor("iden", [P, P], f32, kind="ExternalInput")
    y_ext = nc.dram_tensor("y", [T, C], f32, kind="ExternalOutput")
    router_ext = nc.dram_tensor("router", [T, E], f32, kind="ExternalOutput")

    rden_dram = [nc.dram_tensor(f"rden_scratch{kk}", [2, T], f32, kind="Internal")
                 for kk in range(NC)]

    with tile.TileContext(nc) as tc:
        from contextlib import ExitStack
        with ExitStack() as ctx:
            const = ctx.enter_context(tc.tile_pool(name="const", bufs=1))
            persist = ctx.enter_context(tc.tile_pool(name="persist", bufs=1))
            small = ctx.enter_context(tc.tile_pool(name="small", bufs=4))
            ntmp = ctx.enter_context(tc.tile_pool(name="ntmp", bufs=2))
            ps_mm = ctx.enter_context(tc.tile_pool(name="ps_mm", bufs=2, space="PSUM"))
            ps_tp = ctx.enter_context(tc.tile_pool(name="ps_tp", bufs=2, space="PSUM"))

            # ---- constants ----
            iden = const.tile([P, P], f32, name="iden")
            nc.sync.dma_start(out=iden[:], in_=iden_ext.ap())
            eps_t = const.tile([P, 1], f32, name="eps")
            nc.vector.memset(eps_t[:], EPS)
            ones8 = const.tile([P, H], f32, name="ones8")
            nc.vector.memset(ones8[:], 1.0)
            iden_r = const.tile([P, P], f32r, name="idenr")
            nc.gpsimd.dma_start(out=iden_r[:], in_=iden_ext.ap())

            # persistent activations
            x_sb = [persist.tile([P, C], f32, name=f"x{m}") for m in range(NT)]
            attT = [persist.tile([P, T], f32r, name=f"attT{k}") for k in range(NC)]
            xn2T_r = [persist.tile([P, T], bf16, name=f"xn2Tr{k}") for k in range(NC)]
            router_sb = [persist.tile([P, E], f32, name=f"rt{m}") for m in range(NT)]

            for m in range(NT):
                nc.sync.dma_start(out=x_sb[m][:], in_=x_ext.ap()[m * P:(m + 1) * P, :])

            # O-proj / gate weights loaded early so stage D can start the moment
            # attT is normalized (no SBUF-address-reuse dependency on the
            # attention scope close)
            woT_sb = []
            for kk in range(NC):
                w = persist.tile([P, C], f32r, name=f"wo{kk}")
                nc.gpsimd.dma_start(out=w[:], in_=woT_ext.ap()[kk])
                woT_sb.append(w)

            def rms_scale(xin, tag):
                """returns [P,1] f32 tile = 1/sqrt(mean(xin^2)+eps)"""
                sq = ntmp.tile([P, C], f32, name="sq")
                nc.vector.tensor_mul(out=sq[:], in0=xin, in1=xin)
                ssq = small.tile([P, 1], f32, name="ssq")
                nc.vector.tensor_reduce(out=ssq[:], in_=sq[:],
                                        axis=mybir.AxisListType.X, op=mybir.AluOpType.add)
                rms = small.tile([P, 1], f32, name="rms")
                nc.scalar.activation(out=rms[:], in_=ssq[:],
                                     func=mybir.ActivationFunctionType.Sqrt,
                                     bias=eps_t[:], scale=1.0 / C)
                rsc = small.tile([P, 1], f32, name="rsc")
                nc.vector.reciprocal(out=rsc[:], in_=rms[:])
                return rsc

            # =========================================================
            # attention scope
            # =========================================================
            with ExitStack() as actx:
                apool = actx.enter_context(tc.tile_pool(name="apool", bufs=1))
                expp = actx.enter_context(tc.tile_pool(name="expp", bufs=3))
                ps_sc = actx.enter_context(tc.tile_pool(name="ps_sc", bufs=2, space="PSUM"))
                ps_pv = actx.enter_context(tc.tile_pool(name="ps_pv", bufs=2, space="PSUM"))

                maskT = []
                for t in range(4):
                    mk = apool.tile([P, TCH], f32r, name=f"mask{t}")
                    nc.gpsimd.dma_start(out=mk[:], in_=maskT_ext.ap()[t])
                    maskT.append(mk)

                # ---- stage A: rms_norm1 + transpose -> xnT (f32r) ----
                bctx = ExitStack()
                bpool = bctx.enter_context(tc.tile_pool(name="bpool", bufs=1))
                xnT = [bpool.tile([P, T], f32r, name=f"xnT{k}") for k in range(NC)]
                for m in range(NT):
                    rsc = rms_scale(x_sb[m][:], f"n1_{m}")
                    xn = ntmp.tile([P, C], f32, name="xn")
                    nc.vector.tensor_scalar_mul(out=xn[:], in0=x_sb[m][:], scalar1=rsc[:])
                    for kk in range(NC):
                        tp = ps_tp.tile([P, P], f32, name="tp")
                        nc.tensor.transpose(tp[:], xn[:, kk * P:(kk + 1) * P], iden[:])
                        nc.scalar.copy(out=xnT[kk][:, m * P:(m + 1) * P], in_=tp[:])

                # ---- stage B: QKV projections ----
                wqT_sb, wkT_sb, wvT_sb = [], [], []
                for kk in range(NC):
                    for lst, ext, nm in ((wqT_sb, wqT_ext, "wq"), (wkT_sb, wkT_ext, "wk"),
                                         (wvT_sb, wvT_ext, "wv")):
                        w = bpool.tile([P, C], f32r, name=f"{nm}{kk}")
                        nc.gpsimd.dma_start(out=w[:], in_=ext.ap()[kk])
                        lst.append(w)

                # q stored per-head, zero-padded to K=128 so the scores matmul
                # can contract over the full 128 partitions (the other head's k
                # rows meet zeros). k stored packed (2 heads per tile).
                zeros512 = apool.tile([P, TCH], f32, name="zeros512")
                nc.vector.memset(zeros512[:], 0.0)
                qz = [apool.tile([P, T], f32r, name=f"qz{h}") for h in range(H)]
                for h in range(H):
                    po = D * (h % 2)
                    zo = D - po  # the half that must be zeroed: 64 if h even else 0
                    for tch in range(NTCH):
                        nc.vector.tensor_copy(
                            out=qz[h][zo:zo + D, tch * TCH:(tch + 1) * TCH],
                            in_=zeros512[0:D, :])
                kT = [apool.tile([P, T], f32r, name=f"kT{o}") for o in range(NC)]
                for o4 in range(NC):
                    for tch in range(NTCH):
                        # K projection (packed)
                        pst = ps_mm.tile([P, TCH], f32, name="mmps")
                        for kk in range(NC):
                            nc.tensor.matmul(
                                pst[:],
                                lhsT=wkT_sb[kk][:, o4 * P:(o4 + 1) * P],
                                rhs=xnT[kk][:, tch * TCH:(tch + 1) * TCH],
                                start=(kk == 0), stop=(kk == NC - 1))
                        nc.scalar.copy(
                            out=kT[o4][:, tch * TCH:(tch + 1) * TCH], in_=pst[:])
                        # Q projection (split per head, placed at matching offset)
                        pst = ps_mm.tile([P, TCH], f32, name="mmps")
                        for kk in range(NC):
                            nc.tensor.matmul(
                                pst[:],
                                lhsT=wqT_sb[kk][:, o4 * P:(o4 + 1) * P],
                                rhs=xnT[kk][:, tch * TCH:(tch + 1) * TCH],
                                start=(kk == 0), stop=(kk == NC - 1))
                        nc.scalar.copy(
                            out=qz[2 * o4][0:D, tch * TCH:(tch + 1) * TCH],
                            in_=pst[0:D, :])
                        nc.scalar.copy(
                            out=qz[2 * o4 + 1][D:P, tch * TCH:(tch + 1) * TCH],
                            in_=pst[D:P, :])

                # V in untransposed layout, interleaved per-head with ones column,
                # zero-padded to a full 128-wide stationary (M=128 matmuls are
                # much faster than M=65 on the fp32r path)
                VW = 128
                zrow = apool.tile([P, VW - D - 1], f32, name="zrow")
                nc.vector.memset(zrow[:], 0.0)
                v_aug = [apool.tile([P, H, VW], f32r, name=f"vaug{m}") for m in range(NT)]
                for m in range(NT):
                    pst = ps_mm.tile([P, C], f32, name="mmps")
                    for kk in range(NC):
                        nc.tensor.matmul(
                            pst[:],
                            lhsT=xnT[kk][:, m * P:(m + 1) * P],
                            rhs=wvT_sb[kk][:],
                            start=(kk == 0), stop=(kk == NC - 1))
                    nc.scalar.copy(
                        out=v_aug[m][:, :, 0:D],
                        in_=pst[:].rearrange("p (h d) -> p h d", h=H))
                    nc.vector.tensor_copy(
                        out=v_aug[m][:, :, D:D + 1].rearrange("p h one -> p (h one)"),
                        in_=ones8[:])
                    for h in range(H):
                        nc.vector.tensor_copy(out=v_aug[m][:, h, D + 1:VW], in_=zrow[:])

                bctx.close()  # free xnT + QKV weights (40KB/partition)
                npool = actx.enter_context(tc.tile_pool(name="npool", bufs=1))

                # ---- stage C: attention, pair-major, ic-outer so each
                # 512-token chunk of a pair normalizes as soon as both heads
                # finish it (shortens the tail gating the O-projection) ----
                for kk2 in range(NC):
                    for ic in range(NTCH):
                        for hh in range(2):
                            h = 2 * kk2 + hh
                            h2 = h // 2
                            po = D * (h % 2)
                            jmax = 4 * (ic + 1)
                            pv = ps_pv.tile([P, TCH], f32, name="pvps")
                            pending_pv = []
                            for jj in range(jmax):
                                diag = jj >= 4 * ic
                                sc = ps_sc.tile([P, TCH], f32, name="scps")
                                nc.tensor.matmul(
                                    sc[:],
                                    lhsT=kT[h2][:, jj * P:(jj + 1) * P],
                                    rhs=qz[h][:, ic * TCH:(ic + 1) * TCH],
                                    start=True, stop=not diag)
                                if diag:
                                    # accumulate -1e30 additive causal mask on PE
                                    nc.tensor.matmul(
                                        sc[:],
                                        lhsT=iden_r[:],
                                        rhs=maskT[jj - 4 * ic][:],
                                        start=False, stop=True)
                                ex = expp.tile([P, TCH], f32r, name="expT")
                                nc.scalar.activation(out=ex[:], in_=sc[:],
                                                     func=mybir.ActivationFunctionType.Exp)
                                pending_pv.append((jj, ex))
                                # emit pv for the PREVIOUS jj so PE has a score
                                # matmul in flight while ACT computes this exp
                                if len(pending_pv) > 1:
                                    pjj, pex = pending_pv.pop(0)
                                    nc.tensor.matmul(
                                        pv[:, :],
                                        lhsT=v_aug[pjj][:, h, :],
                                        rhs=pex[:],
                                        start=(pjj == 0), stop=False)
                            pjj, pex = pending_pv.pop(0)
                            nc.tensor.matmul(
                                pv[:, :],
                                lhsT=v_aug[pjj][:, h, :],
                                rhs=pex[:],
                                start=(pjj == 0), stop=True)
                            # evict raw attention (f32r) + denominator row
                            nc.vector.tensor_copy(
                                out=attT[h2][po:po + D, ic * TCH:(ic + 1) * TCH],
                                in_=pv[0:D, :])
                            den_t = small.tile([1, TCH], f32, name="den")
                            nc.scalar.copy(out=den_t[:], in_=pv[D:D + 1, :])
                            nc.sync.dma_start(
                                out=rden_dram[kk2].ap()[hh:hh + 1, ic * TCH:(ic + 1) * TCH],
                                in_=den_t[:])
                        # normalize this (pair, chunk) immediately
                        bc = npool.tile([P, TCH], f32, name="bcast")
                        for hh in range(2):
                            srcap = rden_dram[kk2].ap()[hh:hh + 1, ic * TCH:(ic + 1) * TCH]
                            bsrc = bass.AP(tensor=srcap.tensor, offset=srcap.offset,
                                           ap=[[0, D]] + srcap.ap[1:])
                            nc.sync.dma_start(out=bc[hh * D:(hh + 1) * D, :], in_=bsrc)
                        bc2 = npool.tile([P, TCH], f32, name="bcrec")
                        scr = npool.tile([P, TCH], f32, name="bcscr")
                        nc.vector.reciprocal_approx_accurate(out=bc2[:], in_=bc[:],
                                                             scratch=scr[:])
                        nc.vector.tensor_tensor(
                            out=attT[kk2][:, ic * TCH:(ic + 1) * TCH],
                            in0=attT[kk2][:, ic * TCH:(ic + 1) * TCH],
                            in1=bc2[:], op=mybir.AluOpType.mult)

                # =========================================================
                # stage D (inside attention scope: no release barrier between
                # the last PV and the O-projection)
                # =========================================================
                gate_bc = []
                for e in range(E):
                    gb = npool.tile([P, C], f32, name=f"gbc{e}")
                    nc.sync.dma_start(out=gb[:], in_=gateBC_ext.ap()[e])
                    gate_bc.append(gb)

                for m in range(NT):
                    pat = ps_mm.tile([P, C], f32, name="mmps")
                    for kk in range(NC):
                        nc.tensor.matmul(
                            pat[:],
                            lhsT=attT[kk][:, m * P:(m + 1) * P],
                            rhs=woT_sb[kk][:],
                            start=(kk == 0), stop=(kk == NC - 1))
                    # x1 = x + att_out (in place into x_sb)
                    nc.vector.tensor_tensor(out=x_sb[m][:], in0=x_sb[m][:], in1=pat[:],
                                            op=mybir.AluOpType.add)
                    rsc = rms_scale(x_sb[m][:], f"n2_{m}")
                    xn2 = ntmp.tile([P, C], f32, name="xn2")
                    nc.vector.tensor_scalar_mul(out=xn2[:], in0=x_sb[m][:], scalar1=rsc[:])
                    for kk in range(NC):
                        tp = ps_tp.tile([P, P], f32, name="tp")
                        nc.tensor.transpose(tp[:], xn2[:, kk * P:(kk + 1) * P], iden[:])
                        nc.scalar.copy(out=xn2T_r[kk][:, m * P:(m + 1) * P], in_=tp[:])

                    # gate logits in exact fp32 on DVE: fused mult + free-axis
                    # reduce against pre-broadcast gate rows
                    glog = small.tile([P, E], f32, name="glog")
                    for e in range(E):
                        gsc = ntmp.tile([P, C], f32, name="sq")
                        nc.vector.scalar_tensor_tensor(
                            out=gsc[:], in0=xn2[:], scalar=1.0, in1=gate_bc[e][:],
                            op0=mybir.AluOpType.mult, op1=mybir.AluOpType.mult,
                            accum_out=glog[:, e:e + 1])
                    g = glog[:]
                    m1 = small.tile([P, 1], f32, name="m1")
                    nc.vector.tensor_reduce(out=m1[:], in_=g, axis=mybir.AxisListType.X,
                                            op=mybir.AluOpType.max)
                    m1n = small.tile([P, 1], f32, name="m1n")
                    nc.vector.tensor_scalar_mul(out=m1n[:], in0=m1[:], scalar1=-1.0)
                    ex = small.tile([P, E], f32, name="gex")
                    nc.scalar.activation(out=ex[:], in_=g,
                                         func=mybir.ActivationFunctionType.Exp,
                                         bias=m1n[:], scale=1.0)
                    eq = small.tile([P, E], f32, name="geq")
                    nc.vector.tensor_scalar(out=eq[:], in0=g, scalar1=m1[:], scalar2=None,
                                            op0=mybir.AluOpType.is_ge)
                    gm = small.tile([P, E], f32, name="ggm")
                    nc.vector.scalar_tensor_tensor(out=gm[:], in0=eq[:], scalar=-1e30,
                                                   in1=g, op0=mybir.AluOpType.mult,
                                                   op1=mybir.AluOpType.add)
                    m2 = small.tile([P, 1], f32, name="m2")
                    nc.vector.tensor_reduce(out=m2[:], in_=gm[:], axis=mybir.AxisListType.X,
                                            op=mybir.AluOpType.max)
                    keep = small.tile([P, E], f32, name="gkeep")
                    nc.vector.tensor_scalar(out=keep[:], in0=g, scalar1=m2[:], scalar2=None,
                                            op0=mybir.AluOpType.is_ge)
                    wun = small.tile([P, E], f32, name="gwun")
                    nc.vector.tensor_tensor(out=wun[:], in0=ex[:], in1=keep[:],
                                            op=mybir.AluOpType.mult)
                    s = small.tile([P, 1], f32, name="gsum")
                    nc.vector.tensor_reduce(out=s[:], in_=wun[:], axis=mybir.AxisListType.X,
                                            op=mybir.AluOpType.add)
                    rs = small.tile([P, 1], f32, name="grs")
                    nc.vector.reciprocal(out=rs[:], in_=s[:])
                    nc.vector.tensor_scalar_mul(out=router_sb[m][:], in0=wun[:], scalar1=rs[:])
                    nc.sync.dma_start(out=router_ext.ap()[m * P:(m + 1) * P, :],
                                      in_=router_sb[m][:])
            # attention + stage D scope closed

            # =========================================================
            # stage E scope: MoE pools
            # =========================================================
            with ExitStack() as dctx:
                wfcp = dctx.enter_context(tc.tile_pool(name="wfcp", bufs=2))
                wprp = dctx.enter_context(tc.tile_pool(name="wprp", bufs=1))
                hp = dctx.enter_context(tc.tile_pool(name="hp", bufs=3))
                ps_o = dctx.enter_context(tc.tile_pool(name="ps_o", bufs=4, space="PSUM"))

                def load_expert(e):
                    fc, pr = [], []
                    for kk in range(NC):
                        w = wfcp.tile([P, F], bf16, name=f"wfc{kk}")
                        nc.sync.dma_start(out=w[:], in_=wfcT_ext.ap()[e, kk])
                        fc.append(w)
                    for k in range(NF):
                        w = wprp.tile([P, C], bf16, name=f"wpr{k}")
                        nc.sync.dma_start(out=w[:], in_=wprojT_ext.ap()[e, k])
                        pr.append(w)
                    return fc, pr

                next_w = load_expert(0)

                # =========================================================
                for e in range(E):
                    wfc_sb, wpr_sb = next_w
                    if e + 1 < E:
                        next_w = load_expert(e + 1)
                    for tch in range(NTCH):
                        po_tiles = [ps_o.tile([P, C], f32, name="moeo")
                                    for i in range(4)]
                        pending = []
                        for k in range(NF):
                            ph = ps_mm.tile([P, TCH], f32, name="mmps")
                            for kk in range(NC):
                                nc.tensor.matmul(
                                    ph[:],
                                    lhsT=wfc_sb[kk][:, k * P:(k + 1) * P],
                                    rhs=xn2T_r[kk][:, tch * TCH:(tch + 1) * TCH],
                                    start=(kk == 0), stop=(kk == NC - 1))
                            hT = hp.tile([P, TCH], bf16, name="hT")
                            nc.scalar.activation(out=hT[:], in_=ph[:],
                                                 func=mybir.ActivationFunctionType.Gelu)
                            pending.append((k, hT))
                            if len(pending) > 1:
                                pk, phT = pending.pop(0)
                                for i in range(4):
                                    nc.tensor.matmul(
                                        po_tiles[i][:],
                                        lhsT=phT[:, i * P:(i + 1) * P],
                                        rhs=wpr_sb[pk][:],
                                        start=(pk == 0), stop=False)
                        pk, phT = pending.pop(0)
                        for i in range(4):
                            nc.tensor.matmul(
                                po_tiles[i][:],
                                lhsT=phT[:, i * P:(i + 1) * P],
                                rhs=wpr_sb[pk][:],
                                start=(pk == 0), stop=True)
                        for i in range(4):
                            m = 4 * tch + i
                            nc.vector.scalar_tensor_tensor(
                                out=x_sb[m][:], in0=po_tiles[i][:],
                                scalar=router_sb[m][:, e:e + 1], in1=x_sb[m][:],
                                op0=mybir.AluOpType.mult, op1=mybir.AluOpType.add)

            for m in range(NT):
                nc.sync.dma_start(out=y_ext.ap()[m * P:(m + 1) * P, :], in_=x_sb[m][:])

    nc.compile()
    return nc


def _prep_inputs(x, ln1_w, ln2_w, wq, wk, wv, wo, gate_w, w_fc, w_proj):
    """Host-side weight preprocessing: fold rms-norm gains + score scale,
    pre-transpose for TensorE (contraction dim on partitions)."""
    f = np.float32
    ln1 = np.asarray(ln1_w, f)
    ln2 = np.asarray(ln2_w, f)
    wqT = np.ascontiguousarray((np.asarray(wq, f) * ln1[None, :]).T).reshape(NC, P, C)
    wkT = np.ascontiguousarray((np.asarray(wk, f) * ln1[None, :]).T / np.sqrt(np.float32(D))).reshape(NC, P, C)
    wvT = np.ascontiguousarray((np.asarray(wv, f) * ln1[None, :]).T).reshape(NC, P, C)
    woT = np.ascontiguousarray(np.asarray(wo, f).T).reshape(NC, P, C)
    gateBC = np.ascontiguousarray(
        np.repeat((np.asarray(gate_w, f) * ln2[None, :])[:, None, :], P, axis=1))
    wfcT = np.stack([
        np.ascontiguousarray((np.asarray(w_fc, f)[e] * ln2[None, :]).T).reshape(NC, P, F)
        for e in range(E)]).astype(ml_dtypes.bfloat16)
    wprojT = np.stack([
        np.ascontiguousarray(np.asarray(w_proj, f)[e].T).reshape(NF, P, C)
        for e in range(E)]).astype(ml_dtypes.bfloat16)
    # additive causal masks in transposed-scores layout: tile covers j in
    # [128t, 128t+128), i in [0, 512) relative; keep (0.0) iff
    # j_local <= i_local - 128t else -1e30 (exp -> 0)
    maskT = np.zeros((4, P, TCH), f)
    jl = np.arange(P)[:, None]
    il = np.arange(TCH)[None, :]
    for t in range(4):
        maskT[t] = np.where(jl <= il - P * t, f(0.0), f(-1e30))
    iden = np.eye(P, dtype=f)
    return dict(wqT=wqT, wkT=wkT, wvT=wvT, woT=woT, gateBC=gateBC,
                wfcT=wfcT, wprojT=wprojT, maskT=maskT, iden=iden)


_cached_nc = None


def kernel(x, ln1_w, ln2_w, wq, wk, wv, wo, gate_w, w_fc, w_proj):
    global _cached_nc
    x = np.asarray(x, np.float32)
    B = x.shape[0]
    assert x.shape == (B, T, C) and B == 8
    weights = _prep_inputs(x, ln1_w, ln2_w, wq, wk, wv, wo, gate_w, w_fc, w_proj)
    in_maps = [dict(weights, x=np.ascontiguousarray(x[b])) for b in range(B)]
    if _cached_nc is None:
        _cached_nc = build_kernel()
    res = None
    last_err = None
    for attempt in range(3):
        try:
            res = bass_utils.run_bass_kernel_spmd(_cached_nc, in_maps,
                                                  core_ids=list(range(8)))
            break
        except Exception as err:  # transient NRT device errors: retry
            last_err = err
    if res is None:
        raise last_err
    y = np.stack([res.results[b]["y"] for b in range(B)])
    router = np.concatenate([res.results[b]["router"] for b in range(B)], axis=0)
    return y, router


# revision 24
# speedup vs baseline: 1.0864x; 1.0011x over previous
"""Trainium2 Bass kernel for nn_Block_47004122087942 (attention + dense top-2 MoE).

Strategy: pure data-parallel over batch (B=8 -> 8 NeuronCores, zero collectives).
Per core: one batch element [T=1024, C=512].
Compute dtype: float32r (fp32 storage, TF32-like matmul @ full PE rate) everywhere
except the gate-logit path, which runs in exact fp32 to reproduce the reference's
discrete top-2 expert selection.

Layout convention: "T" suffix = transposed [channels-on-partitions, tokens-on-free].
matmul(out[M,N], lhsT[K,M], rhs[K,N]) computes lhsT.T @ rhs with K = partitions.
"""
import numpy as np
import ml_dtypes

import concourse.bass as bass
import concourse.tile as tile
from concourse import bacc, mybir
from concourse import bass_utils

P = 128
T = 1024          # tokens per core
C = 512           # channels
H = 8             # heads
D = 64            # head dim
E = 4             # experts
F = 2048          # ffn dim
NT = T // P       # 8 token tiles
NC = C // P       # 4 channel chunks
NF = F // P       # 16 ffn tiles
TCH = 512         # matmul free-dim chunk of tokens
NTCH = T // TCH   # 2
EPS = 1e-5

f32 = mybir.dt.float32
f32r = mybir.dt.float32r
bf16 = mybir.dt.bfloat16


def build_kernel():
    nc = bacc.Bacc("TRN2", target_bir_lowering=False, debug=False, num_devices=8)

    # ---- DRAM parameters (per-core shard views) ----
    x_ext = nc.dram_tensor("x", [T, C], f32, kind="ExternalInput")
    wqT_ext = nc.dram_tensor("wqT", [NC, P, C], f32, kind="ExternalInput")
    wkT_ext = nc.dram_tensor("wkT", [NC, P, C], f32, kind="ExternalInput")
    wvT_ext = nc.dram_tensor("wvT", [NC, P, C], f32, kind="ExternalInput")
    woT_ext = nc.dram_tensor("woT", [NC, P, C], f32, kind="ExternalInput")
    gateBC_ext = nc.dram_tensor("gateBC", [E, P, C], f32, kind="ExternalInput")
    wfcT_ext = nc.dram_tensor("wfcT", [E, NC, P, F], bf16, kind="ExternalInput")
    wprojT_ext = nc.dram_tensor("wprojT", [E, NF, P, C], bf16, kind="ExternalInput")
    maskT_ext = nc.dram_tensor("maskT", [4, P, TCH], f32, kind="ExternalInput")
    iden_ext = nc.dram_tensor("iden", [P, P], f32, kind="ExternalInput")
    y_ext = nc.dram_tensor("y", [T, C], f32, kind="ExternalOutput")
    router_ext = nc.dram_tensor("router", [T, E], f32, kind="ExternalOutput")

    rden_dram = [nc.dram_tensor(f"rden_scratch{kk}", [2, T], f32, kind="Internal")
                 for kk in range(NC)]

    with tile.TileContext(nc) as tc:
        from contextlib import ExitStack
        with ExitStack() as ctx:
            const = ctx.enter_context(tc.tile_pool(name="const", bufs=1))
            persist = ctx.enter_context(tc.tile_pool(name="persist", bufs=1))
            small = ctx.enter_context(tc.tile_pool(name="small", bufs=4))
            ntmp = ctx.enter_context(tc.tile_pool(name="ntmp", bufs=2))
            ps_mm = ctx.enter_context(tc.tile_pool(name="ps_mm", bufs=2, space="PSUM"))
            ps_tp = ctx.enter_context(tc.tile_pool(name="ps_tp", bufs=2, space="PSUM"))

            # ---- constants ----
            iden = const.tile([P, P], f32, name="iden")
            nc.sync.dma_start(out=iden[:], in_=iden_ext.ap())
            eps_t = const.tile([P, 1], f32, name="eps")
            nc.vector.memset(eps_t[:], EPS)
            ones8 = const.tile([P, H], f32, name="ones8")
            nc.vector.memset(ones8[:], 1.0)
            iden_r = const.tile([P, P], f32r, name="idenr")
            nc.gpsimd.dma_start(out=iden_r[:], in_=iden_ext.ap())

            # persistent activations
            x_sb = [persist.tile([P, C], f32, name=f"x{m}") for m in range(NT)]
            attT = [persist.tile([P, T], f32r, name=f"attT{k}") for k in range(NC)]
            xn2T_r = [persist.tile([P, T], bf16, name=f"xn2Tr{k}") for k in range(NC)]
            router_sb = [persist.tile([P, E], f32, name=f"rt{m}") for m in range(NT)]

            for m in range(NT):
                nc.sync.dma_start(out=x_sb[m][:], in_=x_ext.ap()[m * P:(m + 1) * P, :])

            # O-proj / gate weights loaded early so stage D can start the moment
            # attT is normalized (no SBUF-address-reuse dependency on the
            # attention scope close)
            woT_sb = []
            for kk in range(NC):
                w = persist.tile([P, C], f32r, name=f"wo{kk}")
                nc.gpsimd.dma_start(out=w[:], in_=woT_ext.ap()[kk])
                woT_sb.append(w)

            def rms_scale(xin, tag):
                """returns [P,1] f32 tile = 1/sqrt(mean(xin^2)+eps)"""
                sq = ntmp.tile([P, C], f32, name="sq")
                nc.vector.tensor_mul(out=sq[:], in0=xin, in1=xin)
                ssq = small.tile([P, 1], f32, name="ssq")
                nc.vector.tensor_reduce(out=ssq[:], in_=sq[:],
                                        axis=mybir.AxisListType.X, op=mybir.AluOpType.add)
                rms = small.tile([P, 1], f32, name="rms")
                nc.scalar.activation(out=rms[:], in_=ssq[:],
                                     func=mybir.ActivationFunctionType.Sqrt,
                                     bias=eps_t[:], scale=1.0 / C)
                rsc = small.tile([P, 1], f32, name="rsc")
                nc.vector.reciprocal(out=rsc[:], in_=rms[:])
                return rsc

            # =========================================================
            # attention scope
            # =========================================================
            with ExitStack() as actx:
                apool = actx.enter_context(tc.tile_pool(name="apool", bufs=1))
                expp = actx.enter_context(tc.tile_pool(name="expp", bufs=3))
                ps_sc = actx.enter_context(tc.tile_pool(name="ps_sc", bufs=2, space="PSUM"))
                ps_pv = actx.enter_context(tc.tile_pool(name="ps_pv", bufs=2, space="PSUM"))

                maskT = []
                for t in range(4):
                    mk = apool.tile([P, TCH], f32r, name=f"mask{t}")
                    nc.gpsimd.dma_start(out=mk[:], in_=maskT_ext.ap()[t])
                    maskT.append(mk)

                # ---- stage A: rms_norm1 + transpose -> xnT (f32r) ----
                bctx = ExitStack()
                bpool = bctx.enter_context(tc.tile_pool(name="bpool", bufs=1))
                xnT = [bpool.tile([P, T], f32r, name=f"xnT{k}") for k in range(NC)]
                for m in range(NT):
                    rsc = rms_scale(x_sb[m][:], f"n1_{m}")
                    xn = ntmp.tile([P, C], f32, name="xn")
                    nc.vector.tensor_scalar_mul(out=xn[:], in0=x_sb[m][:], scalar1=rsc[:])
                    for kk in range(NC):
                        tp = ps_tp.tile([P, P], f32, name="tp")
                        nc.tensor.transpose(tp[:], xn[:, kk * P:(kk + 1) * P], iden[:])
                        nc.scalar.copy(out=xnT[kk][:, m * P:(m + 1) * P], in_=tp[:])

                # ---- stage B: QKV projections ----
                wqT_sb, wkT_sb, wvT_sb = [], [], []
                for kk in range(NC):
                    for lst, ext, nm in ((wqT_sb, wqT_ext, "wq"), (wkT_sb, wkT_ext, "wk"),
                                         (wvT_sb, wvT_ext, "wv")):
                        w = bpool.tile([P, C], f32r, name=f"{nm}{kk}")
                        nc.gpsimd.dma_start(out=w[:], in_=ext.ap()[kk])
                        lst.append(w)

                # q stored per-head, zero-padded to K=128 so the scores matmul
                # can contract over the full 128 partitions (the other head's k
                # rows meet zeros). k stored packed (2 heads per tile).
                zeros512 = apool.tile([P, TCH], f32, name="zeros512")
                nc.vector.memset(zeros512[:], 0.0)
                qz = [apool.tile([P, T], f32r, name=f"qz{h}") for h in range(H)]
                for h in range(H):
                    po = D * (h % 2)
                    zo = D - po  # the half that must be zeroed: 64 if h even else 0
                    for tch in range(NTCH):
                        nc.vector.tensor_copy(
                            out=qz[h][zo:zo + D, tch * TCH:(tch + 1) * TCH],
                            in_=zeros512[0:D, :])
                kT = [apool.tile([P, T], f32r, name=f"kT{o}") for o in range(NC)]
                for o4 in range(NC):
                    for tch in range(NTCH):
                        # K projection (packed)
                        pst = ps_mm.tile([P, TCH], f32, name="mmps")
                        for kk in range(NC):
                            nc.tensor.matmul(
                                pst[:],
                                lhsT=wkT_sb[kk][:, o4 * P:(o4 + 1) * P],
                                rhs=xnT[kk][:, tch * TCH:(tch + 1) * TCH],
                                start=(kk == 0), stop=(kk == NC - 1))
                        nc.scalar.copy(
                            out=kT[o4][:, tch * TCH:(tch + 1) * TCH], in_=pst[:])
                        # Q projection (split per head, placed at matching offset)
                        pst = ps_mm.tile([P, TCH], f32, name="mmps")
                        for kk in range(NC):
                            nc.tensor.matmul(
                                pst[:],
                                lhsT=wqT_sb[kk][:, o4 * P:(o4 + 1) * P],
                                rhs=xnT[kk][:, tch * TCH:(tch + 1) * TCH],
                                start=(kk == 0), stop=(kk == NC - 1))
                        nc.scalar.copy(
                            out=qz[2 * o4][0:D, tch * TCH:(tch + 1) * TCH],
                            in_=pst[0:D, :])
                        nc.scalar.copy(
                            out=qz[2 * o4 + 1][D:P, tch * TCH:(tch + 1) * TCH],
                            in_=pst[D:P, :])

                # V in untransposed layout, interleaved per-head with ones column,
                # zero-padded to a full 128-wide stationary (M=128 matmuls are
                # much faster than M=65 on the fp32r path)
                VW = 128
                zrow = apool.tile([P, VW - D - 1], f32, name="zrow")
                nc.vector.memset(zrow[:], 0.0)
                v_aug = [apool.tile([P, H, VW], f32r, name=f"vaug{m}") for m in range(NT)]
                for m in range(NT):
                    pst = ps_mm.tile([P, C], f32, name="mmps")
                    for kk in range(NC):
                        nc.tensor.matmul(
                            pst[:],
                            lhsT=xnT[kk][:, m * P:(m + 1) * P],
                            rhs=wvT_sb[kk][:],
                            start=(kk == 0), stop=(kk == NC - 1))
                    nc.scalar.copy(
                        out=v_aug[m][:, :, 0:D],
                        in_=pst[:].rearrange("p (h d) -> p h d", h=H))
                    nc.vector.tensor_copy(
                        out=v_aug[m][:, :, D:D + 1].rearrange("p h one -> p (h one)"),
                        in_=ones8[:])
                    for h in range(H):
                        nc.vector.tensor_copy(out=v_aug[m][:, h, D + 1:VW], in_=zrow[:])

                bctx.close()  # free xnT + QKV weights (40KB/partition)
                npool = actx.enter_context(tc.tile_pool(name="npool", bufs=1))

                # ---- stage C: attention, pair-major, ic-outer so each
                # 512-token chunk of a pair normalizes as soon as both heads
                # finish it (shortens the tail gating the O-projection) ----
                for kk2 in range(NC):
                    for ic in range(NTCH):
                        for hh in range(2):
                            h = 2 * kk2 + hh
                            h2 = h // 2
                            po = D * (h % 2)
                            jmax = 4 * (ic + 1)
                            pv = ps_pv.tile([P, TCH], f32, name="pvps")
                            pending_pv = []
                            for jj in range(jmax):
                                diag = jj >= 4 * ic
                                sc = ps_sc.tile([P, TCH], f32, name="scps")
                                nc.tensor.matmul(
                                    sc[:],
                                    lhsT=kT[h2][:, jj * P:(jj + 1) * P],
                                    rhs=qz[h][:, ic * TCH:(ic + 1) * TCH],
                                    start=True, stop=not diag)
                                if diag:
                                    # accumulate -1e30 additive causal mask on PE
                                    nc.tensor.matmul(
                                        sc[:],
                                        lhsT=iden_r[:],
                                        rhs=maskT[jj - 4 * ic][:],
                                        start=False, stop=True)
                                ex = expp.tile([P, TCH], f32r, name="expT")
                                nc.scalar.activation(out=ex[:], in_=sc[:],
                                                     func=mybir.ActivationFunctionType.Exp)
                                pending_pv.append((jj, ex))
                                # emit pv for the PREVIOUS jj so PE has a score
                                # matmul in flight while ACT computes this exp
                                if len(pending_pv) > 1:
                                    pjj, pex = pending_pv.pop(0)
                                    nc.tensor.matmul(
                                        pv[:, :],
                                        lhsT=v_aug[pjj][:, h, :],
                                        rhs=pex[:],
                                        start=(pjj == 0), stop=False)
                            pjj, pex = pending_pv.pop(0)
                            nc.tensor.matmul(
                                pv[:, :],
                                lhsT=v_aug[pjj][:, h, :],
                                rhs=pex[:],
                                start=(pjj == 0), stop=True)
                            # evict raw attention (f32r) + denominator row
                            nc.vector.tensor_copy(
                                out=attT[h2][po:po + D, ic * TCH:(ic + 1) * TCH],
                                in_=pv[0:D, :])
                            den_t = small.tile([1, TCH], f32, name="den")
                            nc.scalar.copy(out=den_t[:], in_=pv[D:D + 1, :])
                            nc.sync.dma_start(
                                out=rden_dram[kk2].ap()[hh:hh + 1, ic * TCH:(ic + 1) * TCH],
                                in_=den_t[:])
                        # normalize this (pair, chunk) immediately
                        bc = npool.tile([P, TCH], f32, name="bcast")
                        for hh in range(2):
                            srcap = rden_dram[kk2].ap()[hh:hh + 1, ic * TCH:(ic + 1) * TCH]
                            bsrc = bass.AP(tensor=srcap.tensor, offset=srcap.offset,
                                           ap=[[0, D]] + srcap.ap[1:])
                            nc.sync.dma_start(out=bc[hh * D:(hh + 1) * D, :], in_=bsrc)
                        bc2 = npool.tile([P, TCH], f32, name="bcrec")
                        scr = npool.tile([P, TCH], f32, name="bcscr")
                        nc.vector.reciprocal_approx_accurate(out=bc2[:], in_=bc[:],
                                                             scratch=scr[:])
                        nc.vector.tensor_tensor(
                            out=attT[kk2][:, ic * TCH:(ic + 1) * TCH],
                            in0=attT[kk2][:, ic * TCH:(ic + 1) * TCH],
                            in1=bc2[:], op=mybir.AluOpType.mult)

                # =========================================================
                # stage D (inside attention scope: no release barrier between
                # the last PV and the O-projection)
                # =========================================================
                gate_bc = []
                for e in range(E):
                    gb = npool.tile([P, C], f32, name=f"gbc{e}")
                    nc.sync.dma_start(out=gb[:], in_=gateBC_ext.ap()[e])
                    gate_bc.append(gb)

                for m in range(NT):
                    pat = ps_mm.tile([P, C], f32, name="mmps")
                    for kk in range(NC):
                        nc.tensor.matmul(
                            pat[:],
                            lhsT=attT[kk][:, m * P:(m + 1) * P],
                            rhs=woT_sb[kk][:],
                            start=(kk == 0), stop=(kk == NC - 1))
                    # x1 = x + att_out (in place into x_sb)
                    nc.vector.tensor_tensor(out=x_sb[m][:], in0=x_sb[m][:], in1=pat[:],
                                            op=mybir.AluOpType.add)
                    rsc = rms_scale(x_sb[m][:], f"n2_{m}")
                    xn2 = ntmp.tile([P, C], f32, name="xn2")
                    nc.vector.tensor_scalar_mul(out=xn2[:], in0=x_sb[m][:], scalar1=rsc[:])
                    for kk in range(NC):
                        tp = ps_tp.tile([P, P], f32, name="tp")
                        nc.tensor.transpose(tp[:], xn2[:, kk * P:(kk + 1) * P], iden[:])
                        nc.scalar.copy(out=xn2T_r[kk][:, m * P:(m + 1) * P], in_=tp[:])

                    # gate logits in exact fp32 on DVE: fused mult + free-axis
                    # reduce against pre-broadcast gate rows
                    glog = small.tile([P, E], f32, name="glog")
                    for e in range(E):
                        gsc = ntmp.tile([P, C], f32, name="sq")
                        nc.vector.scalar_tensor_tensor(
                            out=gsc[:], in0=xn2[:], scalar=1.0, in1=gate_bc[e][:],
                            op0=mybir.AluOpType.mult, op1=mybir.AluOpType.mult,
                            accum_out=glog[:, e:e + 1])
                    g = glog[:]
                    m1 = small.tile([P, 1], f32, name="m1")
                    nc.vector.tensor_reduce(out=m1[:], in_=g, axis=mybir.AxisListType.X,
                                            op=mybir.AluOpType.max)
                    m1n = small.tile([P, 1], f32, name="m1n")
                    nc.vector.tensor_scalar_mul(out=m1n[:], in0=m1[:], scalar1=-1.0)
                    ex = small.tile([P, E], f32, name="gex")
                    nc.scalar.activation(out=ex[:], in_=g,
                                         func=mybir.ActivationFunctionType.Exp,
                                         bias=m1n[:], scale=1.0)
                    eq = small.tile([P, E], f32, name="geq")
                    nc.vector.tensor_scalar(out=eq[:], in0=g, scalar1=m1[:], scalar2=None,
                                            op0=mybir.AluOpType.is_ge)
                    gm = small.tile([P, E], f32, name="ggm")
                    nc.vector.scalar_tensor_tensor(out=gm[:], in0=eq[:], scalar=-1e30,
                                                   in1=g, op0=mybir.AluOpType.mult,
                                                   op1=mybir.AluOpType.add)
                    m2 = small.tile([P, 1], f32, name="m2")
                    nc.vector.tensor_reduce(out=m2[:], in_=gm[:], axis=mybir.AxisListType.X,
                                            op=mybir.AluOpType.max)
                    keep = small.tile([P, E], f32, name="gkeep")
                    nc.vector.tensor_scalar(out=keep[:], in0=g, scalar1=m2[:], scalar2=None,
                                            op0=mybir.AluOpType.is_ge)
                    wun = small.tile([P, E], f32, name="gwun")
                    nc.vector.tensor_tensor(out=wun[:], in0=ex[:], in1=keep[:],
                                            op=mybir.AluOpType.mult)
                    s = small.tile([P, 1], f32, name="gsum")
                    nc.vector.tensor_reduce(out=s[:], in_=wun[:], axis=mybir.AxisListType.X,
                                            op=mybir.AluOpType.add)
                    rs = small.tile([P, 1], f32, name="grs")
                    nc.vector.reciprocal(out=rs[:], in_=s[:])
                    nc.vector.tensor_scalar_mul(out=router_sb[m][:], in0=wun[:], scalar1=rs[:])
                    nc.sync.dma_start(out=router_ext.ap()[m * P:(m + 1) * P, :],
                                      in_=router_sb[m][:])
            # attention + stage D scope closed

            # =========================================================
            # stage E scope: MoE pools
            # =========================================================
            with ExitStack() as dctx:
                wfcp = dctx.enter_context(tc.tile_pool(name="wfcp", bufs=2))
                wprp = dctx.enter_context(tc.tile_pool(name="wprp", bufs=2))
                hp = dctx.enter_context(tc.tile_pool(name="hp", bufs=3))
                ps_o = dctx.enter_context(tc.tile_pool(name="ps_o", bufs=4, space="PSUM"))

                def load_expert(e):
                    fc, pr = [], []
                    for kk in range(NC):
                        w = wfcp.tile([P, F], bf16, name=f"wfc{kk}")
                        nc.sync.dma_start(out=w[:], in_=wfcT_ext.ap()[e, kk])
                        fc.append(w)
                    for k in range(NF):
                        w = wprp.tile([P, C], bf16, name=f"wpr{k}")
                        nc.sync.dma_start(out=w[:], in_=wprojT_ext.ap()[e, k])
                        pr.append(w)
                    return fc, pr

                next_w = load_expert(0)

                # =========================================================
                for e in range(E):
                    wfc_sb, wpr_sb = next_w
                    if e + 1 < E:
                        next_w = load_expert(e + 1)
                    for tch in range(NTCH):
                        po_tiles = [ps_o.tile([P, C], f32, name="moeo")
                                    for i in range(4)]
                        pending = []
                        for k in range(NF):
                            ph = ps_mm.tile([P, TCH], f32, name="mmps")
                            for kk in range(NC):
                                nc.tensor.matmul(
                                    ph[:],
                                    lhsT=wfc_sb[kk][:, k * P:(k + 1) * P],
                                    rhs=xn2T_r[kk][:, tch * TCH:(tch + 1) * TCH],
                                    start=(kk == 0), stop=(kk == NC - 1))
                            hT = hp.tile([P, TCH], bf16, name="hT")
                            nc.scalar.activation(out=hT[:], in_=ph[:],
                                                 func=mybir.ActivationFunctionType.Gelu)
                            pending.append((k, hT))
                            if len(pending) > 1:
                                pk, phT = pending.pop(0)
                                for i in range(4):
                                    nc.tensor.matmul(
                                        po_tiles[i][:],
                                        lhsT=phT[:, i * P:(i + 1) * P],
                                        rhs=wpr_sb[pk][:],
                                        start=(pk == 0), stop=False)
                        pk, phT = pending.pop(0)
                        for i in range(4):
                            nc.tensor.matmul(
                                po_tiles[i][:],
                                lhsT=phT[:, i * P:(i + 1) * P],
                                rhs=wpr_sb[pk][:],
                                start=(pk == 0), stop=True)
                        for i in range(4):
                            m = 4 * tch + i
                            nc.vector.scalar_tensor_tensor(
                                out=x_sb[m][:], in0=po_tiles[i][:],
                                scalar=router_sb[m][:, e:e + 1], in1=x_sb[m][:],
                                op0=mybir.AluOpType.mult, op1=mybir.AluOpType.add)

            for m in range(NT):
                nc.sync.dma_start(out=y_ext.ap()[m * P:(m + 1) * P, :], in_=x_sb[m][:])

    nc.compile()
    return nc


def _prep_inputs(x, ln1_w, ln2_w, wq, wk, wv, wo, gate_w, w_fc, w_proj):
    """Host-side weight preprocessing: fold rms-norm gains + score scale,
    pre-transpose for TensorE (contraction dim on partitions)."""
    f = np.float32
    ln1 = np.asarray(ln1_w, f)
    ln2 = np.asarray(ln2_w, f)
    wqT = np.ascontiguousarray((np.asarray(wq, f) * ln1[None, :]).T).reshape(NC, P, C)
    wkT = np.ascontiguousarray((np.asarray(wk, f) * ln1[None, :]).T / np.sqrt(np.float32(D))).reshape(NC, P, C)
    wvT = np.ascontiguousarray((np.asarray(wv, f) * ln1[None, :]).T).reshape(NC, P, C)
    woT = np.ascontiguousarray(np.asarray(wo, f).T).reshape(NC, P, C)
    gateBC = np.ascontiguousarray(
        np.repeat((np.asarray(gate_w, f) * ln2[None, :])[:, None, :], P, axis=1))
    wfcT = np.stack([
        np.ascontiguousarray((np.asarray(w_fc, f)[e] * ln2[None, :]).T).reshape(NC, P, F)
        for e in range(E)]).astype(ml_dtypes.bfloat16)
    wprojT = np.stack([
        np.ascontiguousarray(np.asarray(w_proj, f)[e].T).reshape(NF, P, C)
        for e in range(E)]).astype(ml_dtypes.bfloat16)
    # additive causal masks in transposed-scores layout: tile covers j in
    # [128t, 128t+128), i in [0, 512) relative; keep (0.0) iff
    # j_local <= i_local - 128t else -1e30 (exp -> 0)
    maskT = np.zeros((4, P, TCH), f)
    jl = np.arange(P)[:, None]
    il = np.arange(TCH)[None, :]
    for t in range(4):
        maskT[t] = np.where(jl <= il - P * t, f(0.0), f(-1e30))
    iden = np.eye(P, dtype=f)
    return dict(wqT=wqT, wkT=wkT, wvT=wvT, woT=woT, gateBC=gateBC,
                wfcT=wfcT, wprojT=wprojT, maskT=maskT, iden=iden)


_cached_nc = None


def kernel(x, ln1_w, ln2_w, wq, wk, wv, wo, gate_w, w_fc, w_proj):
    global _cached_nc
    x = np.asarray(x, np.float32)
    B = x.shape[0]
    assert x.shape == (B, T, C) and B == 8
    weights = _prep_inputs(x, ln1_w, ln2_w, wq, wk, wv, wo, gate_w, w_fc, w_proj)
    in_maps = [dict(weights, x=np.ascontiguousarray(x[b])) for b in range(B)]
    if _cached_nc is None:
        _cached_nc = build_kernel()
    res = None
    last_err = None
    for attempt in range(3):
        try:
            res = bass_utils.run_bass_kernel_spmd(_cached_nc, in_maps,
                                                  core_ids=list(range(8)))
            break
        except Exception as err:  # transient NRT device errors: retry
            last_err = err
    if res is None:
        raise last_err
    y = np.stack([res.results[b]["y"] for b in range(B)])
    router = np.concatenate([res.results[b]["router"] for b in range(B)], axis=0)
    return y, router
